# revision 58
# baseline (speedup 1.0000x reference)
"""Trainium2 Bass kernel for BaseDependentAttentionLayer (GNN message passing).

Strategy (8 NeuronCores, SPMD, no collectives):
  - Nodes sharded by origin: core c owns nodes [c*12500, (c+1)*12500).
  - Every core recomputes the bf16 k|v table for ALL nodes (cheap on PE,
    hides under gather DMA) and q for its own nodes; tables live in DRAM.
  - Edges sharded by origin core, bucketed by (dest-chunk, origin-block),
    padded to 128-edge tiles; per-edge k|v and q fetched with dma_gather
    (int16 indices -> kv table split into 4 chunks of 25088 rows).
  - Segment softmax runs without the max-subtraction pass (shift-invariant;
    values bounded here), so attention reduces to two segment sums that are
    computed with per-tile 0/1 selection-matrix matmuls accumulating into a
    per-128-node-block PSUM tile.  Epilogue (divide, Wo, LayerNorm, residual)
    is fused per block.
"""

import sys

sys.path.insert(0, "/opt/trn_rl_repo")

import numpy as np
import ml_dtypes

import concourse.bass as bass
import concourse.bacc as bacc
import concourse.mybir as mybir
from concourse.tile import TileContext
from concourse.bass_utils import run_bass_kernel_spmd

N = 100000
E = 1600000
D = 64
H = 4
HD = 16
NCORES = 8
NOWN = 12500            # nodes owned per core
NBLK = 98               # 128-node blocks per core
NB = NBLK * 128         # 12544 padded own nodes
NT = 100352             # padded global table rows (= 4 * 25088)
CH = 4                  # dest chunks (int16 gather index limit)
CHROWS = NT // CH       # 25088
GBLK = 3                # node blocks per super-group
LN_EPS = 1e-5
PAD_OID = 200.0         # origin-id sentinel for pad slots (matches no node)

F32 = mybir.dt.float32
BF16 = mybir.dt.bfloat16
I16 = mybir.dt.int16
BF16_NP = ml_dtypes.bfloat16


def _build_structure(origins, dests):
    """Global (core-independent) stream structure + per-core slot data.

    origins/dests: full [E] int arrays.
    Returns (struct, per_core) where struct is identical for all cores.
    """
    owner = origins // NOWN
    per_core_raw = []
    tcnt = np.zeros((NCORES, CH * NBLK), np.int64)
    for c in range(NCORES):
        m = owner == c
        o = (origins[m] - c * NOWN).astype(np.int32)
        d = dests[m].astype(np.int32)
        eids = np.nonzero(m)[0]
        blk = o >> 7
        chunk = d // CHROWS
        cell = chunk * NBLK + blk
        order = np.argsort(cell, kind="stable")
        o, d, eids, cell = o[order], d[order], eids[order], cell[order]
        cnt = np.bincount(cell, minlength=CH * NBLK)
        tcnt[c] = (cnt + 127) // 128
        per_core_raw.append((o, d, eids, cnt))
    cell_tiles = tcnt.max(0)          # [CH*NBLK] tiles per cell, all cores
    cell_tiles = np.maximum(cell_tiles, 1)

    sg_blocks = [list(range(s, min(s + GBLK, NBLK))) for s in range(0, NBLK, GBLK)]
    # stream order: sg -> chunk -> block
    sgs = []
    tile_block = []     # per global tile: block id
    tile_sg = []
    cell_tile_off = np.zeros(CH * NBLK, np.int64)
    t_off = 0
    for blocks in sg_blocks:
        T_c = []
        sg_tiles = []
        for ch in range(CH):
            tc = 0
            for b in blocks:
                cell = ch * NBLK + b
                nt = int(cell_tiles[cell])
                cell_tile_off[cell] = t_off
                tile_block.extend([b] * nt)
                sg_tiles.extend([b] * nt)
                tile_sg.extend([len(sgs)] * nt)
                t_off += nt
                tc += nt
            T_c.append(tc)
        # start/stop flags within the sg
        first = {}
        last = {}
        for i, b in enumerate(sg_tiles):
            if b not in first:
                first[b] = i
            last[b] = i
        sgs.append({
            "blocks": blocks,
            "T_c": T_c,
            "T": sum(T_c),
            "tile_blocks": sg_tiles,
            "first": first,
            "last": last,
            "tile_off": t_off - sum(T_c),
        })
    S_tiles = t_off
    struct = {
        "sgs": sgs,
        "S_tiles": S_tiles,
        "cell_tiles": cell_tiles,
        "cell_tile_off": cell_tile_off,
    }
    return struct, per_core_raw


def _per_core_arrays(struct, core_raw, edge_weights):
    """Build the per-core DRAM-side slot arrays."""
    o, d, eids, cnt = core_raw
    S_tiles = struct["S_tiles"]
    S = S_tiles * 128
    cell_tiles = struct["cell_tiles"]
    cell_tile_off = struct["cell_tile_off"]

    oid = np.full(S, PAD_OID, np.float32)
    kvi = np.zeros(S, np.int16)
    qi = np.zeros(S, np.int16)
    ew4 = np.zeros((S, H), np.float32)

    # place each cell's edges at its slot range
    cell_edge_off = np.zeros(CH * NBLK + 1, np.int64)
    np.cumsum(cnt, out=cell_edge_off[1:])
    for cell in range(CH * NBLK):
        n = int(cnt[cell])
        if n == 0:
            continue
        e0 = cell_edge_off[cell]
        s0 = cell_tile_off[cell] * 128
        ch = cell // NBLK
        sl = slice(s0, s0 + n)
        el = slice(e0, e0 + n)
        oid[sl] = (o[el] & 127).astype(np.float32)
        kvi[sl] = (d[el] - ch * CHROWS).astype(np.int16)
        qi[sl] = o[el].astype(np.int16)
        ew4[sl] = edge_weights[eids[el]] * (HD ** -0.5)

    # tile-major [128, S_tiles(,H)] views (partition = slot % 128)
    oid_t = np.ascontiguousarray(
        oid.reshape(S_tiles, 128).T).astype(BF16_NP)
    ew_t = np.ascontiguousarray(ew4.reshape(S_tiles, 128, H).transpose(1, 0, 2))

    # wrapped int16 index arrays: per-run [16, len/16] replicated to 128 parts
    def wrap(run_vals):
        w = run_vals.reshape(-1, 16).T          # [16, len/16]
        return np.tile(w, (8, 1))               # [128, len/16]

    kvw = np.zeros((128, S_tiles * 8), np.int16)
    qw = np.zeros((128, S_tiles * 8), np.int16)
    for sg in struct["sgs"]:
        t0 = sg["tile_off"]
        qw[:, t0 * 8:(t0 + sg["T"]) * 8] = wrap(qi[t0 * 128:(t0 + sg["T"]) * 128])
        off = t0
        for ch in range(CH):
            tc = sg["T_c"][ch]
            kvw[:, off * 8:(off + tc) * 8] = wrap(kvi[off * 128:(off + tc) * 128])
            off += tc
    return {"oid": oid_t, "ew": ew_t, "kvidx": kvw, "qidx": qw}


def _build_graph(struct, sg_limit=None, dump_tab=False, no_gather=False,
                 gather_only=False):
    nc = bacc.Bacc()
    S_tiles = struct["S_tiles"]

    xT = nc.declare_dram_parameter("xT", [D + 1, NT], BF16, isOutput=False)
    xTo = nc.declare_dram_parameter("xTo", [D + 1, NB], BF16, isOutput=False)
    wkv = nc.declare_dram_parameter("wkv", [D + 1, 2 * D], BF16, isOutput=False)
    wq = nc.declare_dram_parameter("wq", [D + 1, 2 * D], BF16, isOutput=False)
    wot = nc.declare_dram_parameter("wot", [D, D], F32, isOutput=False)
    boc = nc.declare_dram_parameter("boc", [128, D], F32, isOutput=False)
    gam = nc.declare_dram_parameter("gam", [128, D], F32, isOutput=False)
    TMAX = max(sg["T"] for sg in struct["sgs"])
    iot = nc.declare_dram_parameter("iot", [128, 128, TMAX], BF16, isOutput=False)
    idn = nc.declare_dram_parameter("idn", [128, 128], F32, isOutput=False)
    xpb = nc.declare_dram_parameter("xpb", [NB, D], F32, isOutput=False)
    oid = nc.declare_dram_parameter("oid", [128, S_tiles], BF16, isOutput=False)
    ewp = nc.declare_dram_parameter("ewp", [128, S_tiles, H], F32, isOutput=False)
    kvx = nc.declare_dram_parameter("kvx", [128, S_tiles * 8], I16, isOutput=False)
    qx = nc.declare_dram_parameter("qx", [128, S_tiles * 8], I16, isOutput=False)
    out = nc.declare_dram_parameter("out", [NB, D], F32, isOutput=True)

    kv_tab = nc.dram_tensor("kv_tab", [NT, 2 * D], BF16)
    q_tab = nc.dram_tensor("q_tab", [NB, 2 * D], BF16)
    if dump_tab:
        kv_dump = nc.declare_dram_parameter(
            "kv_dump", [1024, 2 * D], BF16, isOutput=True)
        q_dump = nc.declare_dram_parameter(
            "q_dump", [1024, 2 * D], BF16, isOutput=True)
    if gather_only:
        T0 = struct["sgs"][0]["T"]
        g_dump = nc.declare_dram_parameter(
            "g_dump", [128, T0, 2 * D], BF16, isOutput=True)
        g_dump2 = nc.declare_dram_parameter(
            "g_dump2", [128, T0, 2 * D], BF16, isOutput=True)

    with TileContext(nc) as tc:
        with tc.tile_pool(name="const", bufs=1) as cp:
            wkv_t = cp.tile([D + 1, 2 * D], BF16)
            nc.sync.dma_start(out=wkv_t[:], in_=wkv[:])
            wq_t = cp.tile([D + 1, 2 * D], BF16)
            nc.sync.dma_start(out=wq_t[:], in_=wq[:])
            wot_f = cp.tile([D, D], F32)
            nc.sync.dma_start(out=wot_f[:], in_=wot[:])
            wot_t = cp.tile([D, D], BF16)
            nc.vector.tensor_copy(wot_t[:], wot_f[:])
            boc_t = cp.tile([128, D], F32)
            nc.sync.dma_start(out=boc_t[:], in_=boc[:])
            gam_t = cp.tile([128, D], F32)
            nc.sync.dma_start(out=gam_t[:], in_=gam[:])
            iot_t = cp.tile([128, 128, TMAX], BF16)
            nc.sync.dma_start(out=iot_t[:], in_=iot[:])
            idn_t = cp.tile([128, 128], F32)
            nc.sync.dma_start(out=idn_t[:], in_=idn[:])

            # ---- phase 1: build kv table (all nodes) + q table (own nodes)
            SLAB = 16                                          # tiles per slab
            with (
                tc.tile_pool(name="p1sb", bufs=3) as p1,
                tc.tile_pool(name="p1ps", bufs=2, space="PSUM") as p1p,
            ):
                def qkv_slab(src, wt, tab, s, ntile):
                    n0 = s * SLAB * 128
                    xs = p1.tile([D + 1, ntile * 128], BF16, tag="xs")
                    nc.scalar.dma_start(
                        out=xs[:], in_=src[:, n0:n0 + ntile * 128])
                    ps = p1p.tile([128, ntile * 128], F32, tag="ps")
                    for j in range(ntile):
                        nc.tensor.matmul(
                            out=ps[:, j * 128:(j + 1) * 128],
                            lhsT=xs[:, j * 128:(j + 1) * 128],
                            rhs=wt[:],
                            start=True, stop=True)
                    sb = p1.tile([128, ntile, 128], BF16, tag="sb")
                    nc.scalar.copy(
                        sb[:].rearrange("p a d -> p (a d)"),
                        ps[:])
                    nc.sync.dma_start(
                        out=tab[n0:n0 + ntile * 128, :]
                            .rearrange("(a p) d -> p a d", p=128),
                        in_=sb[:])

                for s in range(NT // (SLAB * 128)):            # 49 slabs
                    qkv_slab(xT, wkv_t, kv_tab, s, SLAB)
                nfull_q = NB // (SLAB * 128)                   # 6 slabs
                for s in range(nfull_q):
                    qkv_slab(xTo, wq_t, q_tab, s, SLAB)
                qkv_slab(xTo, wq_t, q_tab, nfull_q,
                         (NB - nfull_q * SLAB * 128) // 128)

            if dump_tab:
                nc.sync.dma_start(out=kv_dump[:], in_=kv_tab[0:1024, :])
                nc.sync.dma_start(out=q_dump[:], in_=q_tab[0:1024, :])
            sgs_run = struct["sgs"] if sg_limit is None else struct["sgs"][:sg_limit]

            # ---- phase 2: edge processing per super-group
            with (
                tc.tile_pool(name="gat", bufs=2) as gp,
                tc.tile_pool(name="met", bufs=2) as mp,
                tc.tile_pool(name="wrk", bufs=2) as wp,
                tc.tile_pool(name="pst", bufs=2) as pp,
                tc.tile_pool(name="bps", bufs=GBLK + 1, space="PSUM") as bp,
                tc.tile_pool(name="tps", bufs=2, space="PSUM") as tp,
                tc.tile_pool(name="ops", bufs=2, space="PSUM") as op,
            ):
                for sg_i, sg in enumerate(sgs_run):
                    T = sg["T"]
                    t0 = sg["tile_off"]
                    kvi_t = mp.tile([128, T * 8], I16, tag="kvi")
                    nc.scalar.dma_start(out=kvi_t[:], in_=kvx[:, t0 * 8:(t0 + T) * 8])
                    qi_t = mp.tile([128, T * 8], I16, tag="qi")
                    nc.scalar.dma_start(out=qi_t[:], in_=qx[:, t0 * 8:(t0 + T) * 8])
                    oid_t = mp.tile([128, T], BF16, tag="oid")
                    nc.sync.dma_start(out=oid_t[:], in_=oid[:, t0:t0 + T])
                    ew_t = mp.tile([128, T, H], F32, tag="ew")
                    nc.sync.dma_start(out=ew_t[:], in_=ewp[:, t0:t0 + T, :])

                    kvg = gp.tile([128, T, 2 * D], BF16, tag="kvg")
                    qg = gp.tile([128, T, 2 * D], BF16, tag="qg")
                    if no_gather:
                        nc.gpsimd.memset(kvg[:], 1.0)
                        nc.gpsimd.memset(qg[:], 1.0)
                    else:
                        off = 0
                        for ch in range(CH):
                            tcn = sg["T_c"][ch]
                            nc.gpsimd.dma_gather(
                                out_ap=kvg[:, off:off + tcn, :],
                                in_ap=kv_tab[ch * CHROWS:(ch + 1) * CHROWS, :],
                                idxs_ap=kvi_t[:, off * 8:(off + tcn) * 8],
                                num_idxs=tcn * 128,
                                num_idxs_reg=tcn * 128,
                                elem_size=2 * D,
                                single_packet=False)
                            off += tcn
                        nc.gpsimd.dma_gather(
                            out_ap=qg[:],
                            in_ap=q_tab[:],
                            idxs_ap=qi_t[:],
                            num_idxs=T * 128,
                            num_idxs_reg=T * 128,
                            elem_size=2 * D,
                            single_packet=False)
                    if gather_only:
                        nc.sync.dma_start(out=g_dump[:], in_=kvg[:])
                        nc.sync.dma_start(out=g_dump2[:], in_=qg[:])
                        continue

                    qk = wp.tile([128, T, D], BF16, tag="qk")
                    nc.vector.tensor_tensor(
                        out=qk[:], in0=qg[:, :, 0:D], in1=kvg[:, :, 0:D],
                        op=mybir.AluOpType.mult)
                    sc = wp.tile([128, T, H], F32, tag="sc")
                    nc.vector.tensor_reduce(
                        out=sc[:],
                        in_=qk[:].rearrange("p t (h d) -> p t h d", h=H),
                        axis=mybir.AxisListType.X, op=mybir.AluOpType.add)
                    ws = wp.tile([128, T, H], F32, tag="ws")
                    nc.vector.tensor_tensor(
                        out=ws[:], in0=sc[:], in1=ew_t[:],
                        op=mybir.AluOpType.mult)
                    ex = wp.tile([128, T, H], BF16, tag="ex")
                    nc.scalar.activation(
                        out=ex[:], in_=ws[:],
                        func=mybir.ActivationFunctionType.Exp)
                    ctb = wp.tile([128, T, D + H], BF16, tag="ctb")
                    nc.vector.tensor_copy(ctb[:, :, D:D + H], ex[:])
                    nc.vector.tensor_tensor(
                        out=ctb[:, :, 0:D].rearrange("p t (e h) -> p t e h", h=H),
                        in0=kvg[:, :, D:2 * D].rearrange("p t (e h) -> p t e h", h=H),
                        in1=ex[:].rearrange("p t (o h) -> p t o h", o=1)
                            .to_broadcast([128, T, HD, H]),
                        op=mybir.AluOpType.mult)
                    sel = wp.tile([128, 128, T], BF16, tag="sel")
                    nc.vector.tensor_tensor(
                        out=sel[:],
                        in0=oid_t[:].rearrange("p (o t) -> p o t", o=1)
                            .to_broadcast([128, 128, T]),
                        in1=iot_t[:, :, 0:T],
                        op=mybir.AluOpType.is_equal)

                    psums = {}
                    for b in sg["blocks"]:
                        psums[b] = bp.tile([128, D + H], F32, tag="bps",
                                           name=f"bps{b}")
                    for i, b in enumerate(sg["tile_blocks"]):
                        nc.tensor.matmul(
                            out=psums[b][:],
                            lhsT=sel[:, :, i],
                            rhs=ctb[:, i, :],
                            start=(sg["first"][b] == i),
                            stop=(sg["last"][b] == i))

                    # ---- epilogue, slabbed over the sg's blocks
                    blocks = sg["blocks"]
                    NBk = len(blocks)
                    b0 = blocks[0]
                    zr = pp.tile([128, NBk, H], F32, tag="zr")
                    vals = pp.tile([128, NBk, D], F32, tag="vals")
                    for i, b in enumerate(blocks):
                        nc.vector.tensor_scalar_add(
                            zr[:, i, :], psums[b][:, D:D + H], 1e-16)
                    nc.vector.reciprocal(zr[:], zr[:])
                    for i, b in enumerate(blocks):
                        nc.vector.tensor_tensor(
                            out=vals[:, i, :].rearrange("p (e h) -> p e h", h=H),
                            in0=psums[b][:, 0:D].rearrange("p (e h) -> p e h", h=H),
                            in1=zr[:, i, :].rearrange("p (o h) -> p o h", o=1)
                                .to_broadcast([128, HD, H]),
                            op=mybir.AluOpType.mult)
                    po = op.tile([128, NBk, D], F32, tag="po")
                    for i in range(NBk):
                        pt = tp.tile([D, 128], F32, tag="pt", name=f"pt{i}")
                        nc.tensor.transpose(out=pt[:], in_=vals[:, i, :],
                                            identity=idn_t[:])
                        vT = pp.tile([D, 128], BF16, tag="vT", name=f"vT{i}")
                        nc.vector.tensor_copy(vT[:], pt[:])
                        nc.tensor.matmul(out=po[:, i, :], lhsT=vT[:], rhs=wot_t[:],
                                         start=True, stop=True)
                    # LayerNorm + residual (slab ops over [128, NBk, D])
                    nmu = pp.tile([128, NBk], F32, tag="nmu")
                    nc.vector.tensor_reduce(
                        out=nmu[:], in_=po[:],
                        axis=mybir.AxisListType.X, op=mybir.AluOpType.add)
                    nc.vector.tensor_scalar_mul(nmu[:], nmu[:], -1.0 / D)
                    ct = pp.tile([128, NBk, D], F32, tag="ct")
                    nc.vector.tensor_tensor(
                        out=ct[:], in0=po[:],
                        in1=nmu[:].rearrange("p (b o) -> p b o", o=1)
                            .to_broadcast([128, NBk, D]),
                        op=mybir.AluOpType.add)
                    nc.gpsimd.tensor_tensor(
                        out=ct[:], in0=ct[:],
                        in1=boc_t[:].rearrange("p (o d) -> p o d", o=1)
                            .to_broadcast([128, NBk, D]),
                        op=mybir.AluOpType.add)
                    sq = pp.tile([128, NBk, D], F32, tag="sq")
                    nc.gpsimd.tensor_tensor(
                        out=sq[:], in0=ct[:], in1=ct[:], op=mybir.AluOpType.mult)
                    v1 = pp.tile([128, NBk], F32, tag="v1")
                    nc.vector.tensor_reduce(
                        out=v1[:], in_=sq[:],
                        axis=mybir.AxisListType.X, op=mybir.AluOpType.add)
                    nc.vector.tensor_scalar(
                        out=v1[:], in0=v1[:],
                        scalar1=1.0 / D, scalar2=LN_EPS,
                        op0=mybir.AluOpType.mult, op1=mybir.AluOpType.add)
                    nc.vector.reciprocal(v1[:], v1[:])
                    rstd = pp.tile([128, NBk], F32, tag="rstd")
                    nc.scalar.sqrt(rstd[:], v1[:])
                    xb = pp.tile([128, NBk, D], F32, tag="xb")
                    nc.sync.dma_start(
                        out=xb[:],
                        in_=xpb[b0 * 128:(b0 + NBk) * 128, :]
                            .rearrange("(a p) d -> p a d", p=128))
                    ot = pp.tile([128, NBk, D], F32, tag="ot")
                    nc.vector.tensor_tensor(
                        out=ot[:], in0=ct[:],
                        in1=rstd[:].rearrange("p (b o) -> p b o", o=1)
                            .to_broadcast([128, NBk, D]),
                        op=mybir.AluOpType.mult)
                    nc.gpsimd.tensor_tensor(
                        out=ot[:], in0=ot[:],
                        in1=gam_t[:].rearrange("p (o d) -> p o d", o=1)
                            .to_broadcast([128, NBk, D]),
                        op=mybir.AluOpType.mult)
                    nc.gpsimd.tensor_tensor(
                        out=ot[:], in0=ot[:], in1=xb[:], op=mybir.AluOpType.add)
                    nc.sync.dma_start(
                        out=out[b0 * 128:(b0 + NBk) * 128, :]
                            .rearrange("(a p) d -> p a d", p=128),
                        in_=ot[:])
    return nc


def kernel(x, edge_index, edge_weights, Wq, bq, Wk, bk, Wv, bv, Wo, bo,
           gamma, beta):
    x = np.asarray(x, np.float32)
    edge_index = np.asarray(edge_index)
    edge_weights = np.asarray(edge_weights, np.float32)
    origins = np.asarray(edge_index[0], np.int64)
    dests = np.asarray(edge_index[1], np.int64)

    struct, per_core_raw = _build_structure(origins, dests)
    nc = _build_graph(struct)
    nc.finalize()

    # shared (replicated) host arrays
    xT = np.zeros((D + 1, NT), np.float32)
    xT[:D, :N] = x.T
    xT[D] = 1.0
    xT = xT.astype(BF16_NP)
    vperm = (np.arange(H)[None, :] * HD + np.arange(HD)[:, None]).ravel()
    wkv = np.zeros((D + 1, 2 * D), np.float32)
    wkv[:D, :D] = np.asarray(Wk, np.float32).T
    wkv[:D, D:] = np.asarray(Wv, np.float32).T[:, vperm]
    wkv[D, :D] = np.asarray(bk, np.float32)
    wkv[D, D:] = np.asarray(bv, np.float32)[vperm]
    wq = np.zeros((D + 1, 2 * D), np.float32)
    wq[:D, :D] = np.asarray(Wq, np.float32).T
    wq[D, :D] = np.asarray(bq, np.float32)
    wkv = wkv.astype(BF16_NP)
    wq = wq.astype(BF16_NP)
    wot = np.ascontiguousarray(np.asarray(Wo, np.float32).T[vperm, :])
    bo = np.asarray(bo, np.float32)
    boc = np.tile((bo - bo.mean())[None, :], (128, 1)).astype(np.float32)
    gam_t = np.tile(np.asarray(gamma, np.float32)[None, :], (128, 1))
    TMAX = max(sg["T"] for sg in struct["sgs"])
    iot = np.tile(np.arange(128, dtype=np.float32)[None, :, None],
                  (128, 1, TMAX)).astype(BF16_NP)
    idn = np.eye(128, dtype=np.float32)

    in_maps = []
    for c in range(NCORES):
        data = _per_core_arrays(struct, per_core_raw[c], edge_weights)
        xTo = np.zeros((D + 1, NB), np.float32)
        xTo[:D, :NOWN] = x[c * NOWN:(c + 1) * NOWN].T
        xTo[D] = 1.0
        xTo = xTo.astype(BF16_NP)
        xpb = np.zeros((NB, D), np.float32)
        xpb[:NOWN] = x[c * NOWN:(c + 1) * NOWN] + np.asarray(beta, np.float32)
        in_maps.append({
            "xT": xT, "xTo": xTo, "wkv": wkv, "wq": wq, "wot": wot,
            "boc": boc, "gam": gam_t, "iot": iot, "idn": idn, "xpb": xpb,
            "oid": data["oid"], "ewp": data["ew"],
            "kvx": data["kvidx"], "qx": data["qidx"],
        })

    global LAST_SIM_NS
    if SIMULATE_COST:
        from concourse import bass_interp
        sim = bass_interp.CoreSim(nc, no_exec=True, publish_trace=False)
        sim.event_loop()
        LAST_SIM_NS = int(sim.time)

    res = run_bass_kernel_spmd(nc, in_maps, core_ids=list(range(NCORES)),
                               trace=TRACE)
    global LAST_RESULT
    LAST_RESULT = res
    outs = [np.asarray(res.results[i]["out"])[:NOWN] for i in range(NCORES)]
    return np.concatenate(outs, axis=0).astype(np.float32)


TRACE = False
SIMULATE_COST = False
LAST_RESULT = None
LAST_SIM_NS = None



# revision 59
# speedup vs baseline: 1.2497x; 1.2497x over previous
"""Trainium2 Bass kernel for BaseDependentAttentionLayer (GNN message passing).

Strategy (8 NeuronCores, SPMD, no collectives):
  - Nodes sharded by origin: core c owns nodes [c*12500, (c+1)*12500).
  - Origins are permuted into 98 blocks of <=128 per core, bin-packed so
    each (dest-chunk, block) cell holds ~<=512 edges -> near-zero tile pad.
  - Each core computes the bf16 [k|v] row table for ALL nodes (4 chunk
    tensors in DRAM so edge gathers overlap the build) and q for its own
    nodes (kept in SBUF).
  - Edges bucketed by (dest-chunk, origin-block), padded to 128-edge tiles;
    per-edge k|v fetched with dma_gather (int16 idx, 256B rows).
  - Per-slot q is NOT gathered: a pair-one-hot table (129x129 rows of
    [onehot(a)|onehot(b)], 512B) is gathered transpose-style to give
    selT[origin, slot]; one matmul per tile against the q block broadcasts
    q to slots (PSUM), copied to SBUF bf16 by ACT.
  - Segment softmax runs without max-subtraction (shift-invariant; values
    bounded); segment sums via 0/1 selection-matrix matmuls into per-block
    PSUM accumulators; sel is built by DVE is_equal at 2 elem/cycle.
  - Scores reduce via a bf16 tree-fold (2x DVE rate). exp writes directly
    into the ctb tail. LayerNorm+residual is deferred to one batched final
    phase; rstd = exp(-0.5*ln(var+eps)) so ACT never swaps tables.
"""

import sys

sys.path.insert(0, "/opt/trn_rl_repo")

import numpy as np
import ml_dtypes

import concourse.bass as bass
import concourse.bacc as bacc
import concourse.mybir as mybir
from concourse.tile import TileContext
from concourse.bass_utils import run_bass_kernel_spmd

N = 100000
E = 1600000
D = 64
H = 4
HD = 16
NCORES = 8
NOWN = 12500            # nodes owned per core
NBLK = 98               # 128-node origin blocks per core
NB = NBLK * 128         # 12544 padded own nodes
NT = 100352             # padded global table rows (= 4 * 25088)
CH = 4                  # dest chunks (int16 gather index limit)
CHROWS = NT // CH       # 25088
TCAP = 64               # max tiles per super-group (psum / sbuf budget)
BCAP = 4                # max blocks per super-group
LN_EPS = 1e-5
PAD_OID = 200.0         # origin-id sentinel for pad slots (matches no node)
POT_SYM = 129           # pair-one-hot symbols (128 origins + zero pad)

F32 = mybir.dt.float32
BF16 = mybir.dt.bfloat16
I16 = mybir.dt.int16
BF16_NP = ml_dtypes.bfloat16

SLAB = 2048             # phase-A rows per slab


def _balance_blocks(chunk_of_edge_dest, local_origin):
    """Assign local origins to NBLK blocks (<=128 each), balancing the
    per-(chunk, block) edge counts with 4-D LPT so cells pack near 512.

    Returns perm: perm[new_node_index] = local_origin (block b owns
    perm[b*128:(b+1)*128] entries; tail entries may be -1 = unused)."""
    cnt = np.zeros((NOWN, CH), np.int64)
    np.add.at(cnt, (local_origin, chunk_of_edge_dest), 1)
    deg = cnt.sum(1)
    order = np.argsort(-deg, kind="stable")
    CAP = 4 * 128            # capped blocks: every cell fits in 4 tiles
    NOVF = 2                 # overflow blocks (uncapped) at the end
    NCAPB = NBLK - NOVF
    bsum = np.zeros((NBLK, CH), np.int64)
    bcnt = np.zeros(NBLK, np.int64)
    assign = np.zeros(NOWN, np.int64)
    # heaviest origins soak into the overflow blocks so the capped blocks'
    # total fits under NCAPB*CAP per chunk
    for i, o in enumerate(order[:NOVF * 128]):
        b = NCAPB + i % NOVF
        assign[o] = b
        bsum[b] += cnt[o]
        bcnt[b] += 1
    for o in order[NOVF * 128:]:
        c = cnt[o]
        cand = (bsum[:NCAPB] + c[None, :]).max(1).astype(np.float64)
        open_ = bcnt[:NCAPB] < 128
        feas = (cand <= CAP) & open_
        if feas.any():
            # worst-fit: keep all blocks growing evenly (tight 4-D packing)
            score = np.where(feas, cand, np.inf)
        else:
            score = np.where(open_, cand, np.inf)
        b = int(np.argmin(score))
        assign[o] = b
        bsum[b] += c
        bcnt[b] += 1
    perm = np.full(NBLK * 128, -1, np.int64)
    pos = 0
    for b in range(NBLK):
        members = np.nonzero(assign == b)[0]
        perm[b * 128:b * 128 + len(members)] = members
    return perm, assign


def _build_structure(origins, dests, edge_weights):
    """Global (core-independent) tile structure + per-core slot data.

    Dest nodes are compacted per core (only referenced nodes get kv-table
    rows); chunking is by compacted rank."""
    owner = origins // NOWN
    # pass 1: per-core unique dests -> table size (shared across cores)
    ucols = []
    for c in range(NCORES):
        ucols.append(np.unique(dests[owner == c]))
    max_u = max(len(u) for u in ucols)
    chrows = -(-max_u // (4 * SLAB)) * SLAB      # per-chunk rows, 2048-mult
    assert chrows <= 32000
    per_core = []
    cell_cnt = np.zeros((NCORES, CH * NBLK), np.int64)
    for c in range(NCORES):
        m = owner == c
        o = (origins[m] - c * NOWN).astype(np.int64)
        d = dests[m].astype(np.int64)
        ew = edge_weights[m]
        rank = np.searchsorted(ucols[c], d)
        # chunk boundaries equalize EDGES per chunk (cells pack to ~512);
        # each chunk's rank span must still fit the CHR-row table
        rdeg = np.bincount(rank, minlength=len(ucols[c]))
        cum = np.cumsum(rdeg)
        nb_ = [0] + [int(np.searchsorted(cum, cum[-1] * k // CH))
                     for k in (1, 2, 3)] + [len(ucols[c])]
        bounds = np.asarray(nb_, np.int64)
        assert (np.diff(bounds) <= chrows).all()
        chunk = np.searchsorted(bounds[1:-1], rank, side="right")
        perm, assign = _balance_blocks(chunk, o)
        # origin -> (block, slot-in-block) position
        opos = np.zeros(NOWN, np.int64)
        valid = perm >= 0
        opos[perm[valid]] = np.nonzero(valid)[0]
        p = opos[o]                       # position in permuted node space
        blk = p >> 7
        cell = chunk * NBLK + blk
        order = np.argsort(cell, kind="stable")
        cnt = np.bincount(cell, minlength=CH * NBLK)
        cell_cnt[c] = cnt
        per_core.append({
            "perm": perm, "cell": cell[order], "oid": (p & 127)[order],
            "dloc": (rank - bounds[chunk])[order], "ew": ew[order],
            "cnt": cnt, "ucol": ucols[c], "bounds": bounds,
        })
    cmax = cell_cnt.max(0)
    cell_tiles = np.maximum((cmax + 127) // 128, 1)   # [CH*NBLK]

    # super-groups: consecutive blocks, <=BCAP blocks, <=TCAP tiles, T even
    blk_tiles = cell_tiles.reshape(CH, NBLK).sum(0)   # tiles per block
    sgs = []
    start = 0
    while start < NBLK:
        nb, t = 0, 0
        while (start + nb < NBLK and nb < BCAP
               and t + blk_tiles[start + nb] <= TCAP):
            t += blk_tiles[start + nb]
            nb += 1
        if nb == 0:
            nb, t = 1, int(blk_tiles[start])
        blocks = list(range(start, start + nb))
        pad_tile = t % 2                # keep T even for the selT gather
        sgs.append({"blocks": blocks, "T": t + pad_tile, "pad_tile": pad_tile})
        start += nb

    # stream order: sg -> chunk -> block; the optional pad tile sits at the
    # end of the last chunk segment and belongs to the sg's first block.
    t_off = 0
    cell_tile_off = np.zeros(CH * NBLK, np.int64)
    for sg in sgs:
        sg["tile_off"] = t_off
        T_c = []
        tile_blocks = []
        for ch in range(CH):
            tc = 0
            for b in sg["blocks"]:
                cell = ch * NBLK + b
                nt = int(cell_tiles[cell])
                cell_tile_off[cell] = t_off
                tile_blocks.extend([b] * nt)
                t_off += nt
                tc += nt
            if ch == CH - 1 and sg["pad_tile"]:
                tile_blocks.append(sg["blocks"][0])
                t_off += 1
                tc += 1
            T_c.append(tc)
        first, last = {}, {}
        for i, b in enumerate(tile_blocks):
            if b not in first:
                first[b] = i
            last[b] = i
        sg["T_c"] = T_c
        sg["tile_blocks"] = tile_blocks
        sg["first"] = first
        sg["last"] = last
    S_tiles = t_off
    struct = {"sgs": sgs, "S_tiles": S_tiles, "cell_tiles": cell_tiles,
              "cell_tile_off": cell_tile_off, "chrows": int(chrows),
              "TMAX": max(sg["T"] for sg in sgs)}
    return struct, per_core


META_W = 21             # int16 units per tile: oid 1 + ew 4 + kvx 8 + qx 8


def _per_core_arrays(struct, core, scale):
    """Packed per-core metadata [128, S_tiles*META_W] int16 plus host perm."""
    S_tiles = struct["S_tiles"]
    S = S_tiles * 128
    cell_tile_off = struct["cell_tile_off"]

    oid = np.full(S, PAD_OID, np.float32)
    kvi = np.zeros(S, np.int16)
    ew4 = np.zeros((S, H), np.float32)
    qxi = np.zeros(S, np.int16)

    cnt = core["cnt"]
    cell_edge_off = np.zeros(CH * NBLK + 1, np.int64)
    np.cumsum(cnt, out=cell_edge_off[1:])
    for cell in range(CH * NBLK):
        n = int(cnt[cell])
        if n == 0:
            continue
        e0 = cell_edge_off[cell]
        s0 = cell_tile_off[cell] * 128
        sl = slice(s0, s0 + n)
        el = slice(e0, e0 + n)
        oid[sl] = core["oid"][el].astype(np.float32)
        kvi[sl] = core["dloc"][el].astype(np.int16)
        ew4[sl] = core["ew"][el] * scale
        qxi[sl] = (core["oid"][el] * NBLK + (cell % NBLK)).astype(np.int16)

    def wrap(run_vals):
        w = run_vals.reshape(-1, 16).T
        return np.tile(w, (8, 1))

    # per-sg contiguous regions: [oid(T) | ew(T*4) | kvx(T*8) | prx(T*4)]
    oid_pt = np.ascontiguousarray(
        oid.reshape(S_tiles, 128).T).astype(BF16_NP).view(np.int16)
    ew_pt = ew4.astype(BF16_NP).view(np.int16) \
        .reshape(S_tiles, 128, H).transpose(1, 0, 2)
    meta = np.zeros((128, S_tiles * META_W), np.int16)
    for sg in struct["sgs"]:
        t0, T = sg["tile_off"], sg["T"]
        m0 = t0 * META_W
        meta[:, m0:m0 + T] = oid_pt[:, t0:t0 + T]
        meta[:, m0 + T:m0 + 5 * T] = \
            ew_pt[:, t0:t0 + T, :].reshape(128, T * H)
        meta[:, m0 + 5 * T:m0 + 13 * T] = \
            wrap(kvi[t0 * 128:(t0 + T) * 128])
        meta[:, m0 + 13 * T:m0 + 21 * T] = \
            wrap(qxi[t0 * 128:(t0 + T) * 128])
    return meta


def _build_graph(struct):
    nc = bacc.Bacc()
    S_tiles = struct["S_tiles"]
    TMAX = struct["TMAX"]
    sgs = struct["sgs"]
    CHR = struct["chrows"]

    # x for the kv table, dest-compacted, with a ones row for the bias
    xT = nc.declare_dram_parameter("xT", [D + 1, CH * CHR], BF16,
                                   isOutput=False)
    xTo = nc.declare_dram_parameter("xTo", [D + 1, NB], BF16, isOutput=False)
    wkv = nc.declare_dram_parameter("wkv", [D + 1, 2 * D], BF16, isOutput=False)
    wq = nc.declare_dram_parameter("wq", [D + 1, 2 * D], BF16, isOutput=False)
    wot = nc.declare_dram_parameter("wot", [D, D], F32, isOutput=False)
    boc = nc.declare_dram_parameter("boc", [128, D], F32, isOutput=False)
    gam = nc.declare_dram_parameter("gam", [128, D], F32, isOutput=False)
    THALF = (TMAX + 1) // 2
    iot = nc.declare_dram_parameter("iot", [128, 128, THALF], BF16, isOutput=False)
    idn = nc.declare_dram_parameter("idn", [128, 128], F32, isOutput=False)
    xpb = nc.declare_dram_parameter("xpb", [NB, D], F32, isOutput=False)
    meta = nc.declare_dram_parameter("meta", [128, S_tiles * META_W], I16,
                                     isOutput=False)
    out = nc.declare_dram_parameter("out", [NB, D], F32, isOutput=True)

    kv_tab = [nc.dram_tensor(f"kv_tab{ch}", [CHR, 2 * D], BF16)
              for ch in range(CH)]
    q_tab = nc.dram_tensor("q_tab", [NB, 2 * D], BF16)

    with TileContext(nc) as tc:
        with tc.tile_pool(name="const", bufs=1) as cp:
            wkv_t = cp.tile([D + 1, 2 * D], BF16)
            nc.sync.dma_start(out=wkv_t[:], in_=wkv[:])
            wq_t = cp.tile([D + 1, 2 * D], BF16)
            nc.sync.dma_start(out=wq_t[:], in_=wq[:])
            wot_f = cp.tile([D, D], F32)
            nc.sync.dma_start(out=wot_f[:], in_=wot[:])
            wot_t = cp.tile([D, D], BF16)
            nc.vector.tensor_copy(wot_t[:], wot_f[:])
            boc_t = cp.tile([128, D], F32)
            nc.sync.dma_start(out=boc_t[:], in_=boc[:])
            gam_t = cp.tile([128, D], F32)
            nc.sync.dma_start(out=gam_t[:], in_=gam[:])
            iot_t = cp.tile([128, 128, THALF], BF16)
            nc.sync.dma_start(out=iot_t[:], in_=iot[:])
            idn_t = cp.tile([128, 128], F32)
            nc.sync.dma_start(out=idn_t[:], in_=idn[:])
            poall = cp.tile([128, NBLK, D], F32)       # post-Wo, pre-LN

            # ---- phase A: q (SBUF) first, then kv chunk tables (DRAM).
            # x comes half-packed [130, rows/2]: even rows in partitions
            # 0..64, odd rows in 65..129 (halves the DMA column count).
            BIG = 8192
            with (
                tc.tile_pool(name="pa", bufs=2) as pa,
                tc.tile_pool(name="paq", bufs=1) as paq,
                tc.tile_pool(name="pap", bufs=2, space="PSUM") as pap,
            ):
                # q: one load; block b = columns [b*128, (b+1)*128)
                xq = paq.tile([D + 1, NB], BF16)
                nc.sync.dma_start(out=xq[:], in_=xTo[:])
                QB = 16                      # blocks per psum slab
                for s in range((NBLK + QB - 1) // QB):
                    b0 = s * QB
                    nblk = min(QB, NBLK - b0)
                    ps = pap.tile([128, (SLAB // 128) * 2 * D], F32, tag="ps",
                                  name=f"psq{s}")
                    psv = ps[:].rearrange("p (a d) -> p a d", d=128)
                    for j in range(nblk):
                        b = b0 + j
                        nc.tensor.matmul(
                            out=psv[:, j, :],
                            lhsT=xq[:, b * 128:(b + 1) * 128],
                            rhs=wq_t[:],
                            start=True, stop=True)
                    qb = pa.tile([128, SLAB // 128, 2 * D], BF16, tag="qb")
                    nc.scalar.copy(
                        qb[:, 0:nblk, :].rearrange("p a d -> p (a d)"),
                        ps[:, 0:nblk * 2 * D])
                    # q_tab row p*NBLK + b (p-major: contiguous per partition)
                    nc.sync.dma_start(
                        out=q_tab[:].rearrange("(p a) d -> p a d", p=128)
                            [:, b0:b0 + nblk, :],
                        in_=qb[:, 0:nblk, :])

                # kv: 8192-row DMA slabs, 2048-row GEMM sub-slabs. Partition
                # p holds table rows [n0+rpp8*p, +rpp8); tile j of sub-slab
                # s2 covers rows {rpp8*p + spp*s2 + j} (p-strided lhsT).
                for ch in range(CH):
                    n0 = 0
                    while n0 < CHR:
                        nrows = min(BIG, CHR - n0)
                        rpp8 = nrows // 128
                        xs = pa.tile([D + 1, BIG], BF16, tag="xs")
                        nc.sync.dma_start(
                            out=xs[:, 0:nrows],
                            in_=xT[:, ch * CHR + n0:ch * CHR + n0 + nrows])
                        sb = pa.tile([128, BIG // 128, 2 * D], BF16, tag="sb")
                        nsub = nrows // SLAB
                        spp = rpp8 // nsub       # rows per partition per sub
                        for s2 in range(nsub):
                            ps = pap.tile([128, (SLAB // 128) * 2 * D], F32,
                                          tag="ps", name=f"pkv{ch}_{n0}_{s2}")
                            for j in range(SLAB // 128):
                                r = spp * s2 + j
                                nc.tensor.matmul(
                                    out=ps[:, j * 2 * D:(j + 1) * 2 * D],
                                    lhsT=xs[:, r:nrows:rpp8],
                                    rhs=wkv_t[:],
                                    start=True, stop=True)
                            # alternate the psum->sbuf copy between ACT and
                            # the (phase-A-idle) DVE to halve the serial span
                            dst = sb[:, spp * s2:spp * (s2 + 1), :] \
                                .rearrange("p a d -> p (a d)")
                            if s2 % 2 == 0:
                                nc.scalar.copy(dst, ps[:, 0:SLAB // 128 * 2 * D])
                            else:
                                nc.vector.tensor_copy(
                                    dst, ps[:, 0:SLAB // 128 * 2 * D])
                        nc.sync.dma_start(
                            out=kv_tab[ch][n0:n0 + nrows, :]
                                .rearrange("(p a) d -> p a d", p=128),
                            in_=sb[:, 0:rpp8, :])
                        n0 += nrows

            # ---- phase B: edge processing per super-group
            with (
                tc.tile_pool(name="gat", bufs=2) as gp,
                tc.tile_pool(name="met", bufs=2) as mp,
                tc.tile_pool(name="wrk", bufs=2) as wp,
                tc.tile_pool(name="dvi", bufs=1) as dv,
                tc.tile_pool(name="eps", bufs=2) as ep,
                tc.tile_pool(name="bps", bufs=6, space="PSUM") as bp,
                tc.tile_pool(name="tpo", bufs=2, space="PSUM") as tp,
            ):
                for sg in sorted(sgs, key=lambda g: -g["T"]):
                    T = sg["T"]
                    t0 = sg["tile_off"]
                    mt = mp.tile([128, TMAX * META_W], I16, tag="mt")
                    nc.sync.dma_start(
                        out=mt[:, 0:T * META_W],
                        in_=meta[:, t0 * META_W:(t0 + T) * META_W])
                    oid_t = mt[:, 0:T].bitcast(BF16)
                    ew_t = mt[:, T:5 * T].bitcast(BF16) \
                        .rearrange("p (t w) -> p t w", w=H)
                    kvi_t = mt[:, 5 * T:13 * T]
                    qxi_t = mt[:, 13 * T:21 * T]

                    kvg = gp.tile([128, TMAX, 2 * D], BF16, tag="kvg")
                    off = 0
                    for ch in range(CH):
                        tcn = sg["T_c"][ch]
                        if tcn == 0:
                            continue
                        nc.gpsimd.dma_gather(
                            out_ap=kvg[:, off:off + tcn, :],
                            in_ap=kv_tab[ch][:],
                            idxs_ap=kvi_t[:, off * 8:(off + tcn) * 8],
                            num_idxs=tcn * 128,
                            num_idxs_reg=tcn * 128,
                            elem_size=2 * D,
                            single_packet=False)
                        off += tcn
                    qg = gp.tile([128, TMAX, 2 * D], BF16, tag="qg")
                    nc.gpsimd.dma_gather(
                        out_ap=qg[:, 0:T, :],
                        in_ap=q_tab[:],
                        idxs_ap=qxi_t[:],
                        num_idxs=T * 128,
                        num_idxs_reg=T * 128,
                        elem_size=2 * D,
                        single_packet=False)

                    # scores: qk mult (2x bf16) then bf16 tree reduce over d
                    qk = dv.tile([128, TMAX, HD, H], BF16, tag="qk")
                    nc.vector.tensor_tensor(
                        out=qk[:, 0:T, :, :],
                        in0=qg[:, 0:T, 0:D].rearrange("p t (e h) -> p t e h", h=H),
                        in1=kvg[:, 0:T, 0:D].rearrange("p t (e h) -> p t e h", h=H),
                        op=mybir.AluOpType.mult)
                    r8 = dv.tile([128, TMAX, 8, H], BF16, tag="r8")
                    nc.vector.tensor_tensor(
                        out=r8[:, 0:T, :, :], in0=qk[:, 0:T, 0:8, :],
                        in1=qk[:, 0:T, 8:16, :], op=mybir.AluOpType.add)
                    r4 = dv.tile([128, TMAX, 4, H], BF16, tag="r4")
                    nc.vector.tensor_tensor(
                        out=r4[:, 0:T, :, :], in0=r8[:, 0:T, 0:4, :],
                        in1=r8[:, 0:T, 4:8, :], op=mybir.AluOpType.add)
                    r2 = dv.tile([128, TMAX, 2, H], BF16, tag="r2")
                    nc.vector.tensor_tensor(
                        out=r2[:, 0:T, :, :], in0=r4[:, 0:T, 0:2, :],
                        in1=r4[:, 0:T, 2:4, :], op=mybir.AluOpType.add)
                    ws = dv.tile([128, TMAX, 1, H], BF16, tag="ws")
                    nc.vector.tensor_tensor(
                        out=ws[:, 0:T, :, :], in0=r2[:, 0:T, 0:1, :],
                        in1=r2[:, 0:T, 1:2, :], op=mybir.AluOpType.add)
                    wsb = dv.tile([128, TMAX, H], BF16, tag="wsb")
                    nc.vector.tensor_tensor(
                        out=wsb[:, 0:T, :], in0=ws[:, 0:T, 0, :],
                        in1=qg[:, 0:T, D:D + H], op=mybir.AluOpType.add)
                    wse = wp.tile([128, TMAX, H], BF16, tag="wse")
                    nc.vector.tensor_tensor(
                        out=wse[:, 0:T, :],
                        in0=wsb[:, 0:T, :],
                        in1=ew_t[:],
                        op=mybir.AluOpType.mult)
                    ctb = wp.tile([128, TMAX, D + H], BF16, tag="ctb")
                    nc.scalar.activation(
                        out=ctb[:, 0:T, D:D + H], in_=wse[:, 0:T, :],
                        func=mybir.ActivationFunctionType.Exp)
                    nc.vector.tensor_tensor(
                        out=ctb[:, 0:T, 0:D].rearrange("p t (e h) -> p t e h", h=H),
                        in0=kvg[:, 0:T, D:2 * D].rearrange("p t (e h) -> p t e h", h=H),
                        in1=ctb[:, 0:T, D:D + H].rearrange("p t (o h) -> p t o h", o=1)
                            .to_broadcast([128, T, HD, H]),
                        op=mybir.AluOpType.mult)

                    # sel one-hots + scatter matmuls into per-block psums
                    # (two instrs so the iota const is only TMAX/2 deep)
                    sel = wp.tile([128, 128, TMAX], BF16, tag="sel")
                    ha = T // 2
                    for s0, sn in ((0, ha), (ha, T - ha)):
                        nc.vector.tensor_tensor(
                            out=sel[:, :, s0:s0 + sn],
                            in0=oid_t[:, s0:s0 + sn]
                                .rearrange("p (o t) -> p o t", o=1)
                                .to_broadcast([128, 128, sn]),
                            in1=iot_t[:, :, 0:sn],
                            op=mybir.AluOpType.is_equal)
                    psums = {}
                    for i, b in enumerate(sg["blocks"]):
                        psums[b] = bp.tile([128, D + H], F32, tag="bps",
                                           name=f"bps{b}")[:]
                    for i, b in enumerate(sg["tile_blocks"]):
                        nc.tensor.matmul(
                            out=psums[b][:],
                            lhsT=sel[:, :, i],
                            rhs=ctb[:, i, :],
                            start=(sg["first"][b] == i),
                            stop=(sg["last"][b] == i))

                    # per-sg epilogue: divide by z, transpose, Wo -> poall
                    blocks = sg["blocks"]
                    NBk = len(blocks)
                    zr = ep.tile([128, BCAP, H], F32, tag="zr")
                    for i, b in enumerate(blocks):
                        nc.vector.tensor_scalar_add(
                            zr[:, i, :], psums[b][:, D:D + H], 1e-16)
                    nc.vector.reciprocal(zr[:, 0:NBk, :], zr[:, 0:NBk, :])
                    vals = ep.tile([128, BCAP, D], F32, tag="vals")
                    for i, b in enumerate(blocks):
                        nc.vector.tensor_tensor(
                            out=vals[:, i, :].rearrange("p (e h) -> p e h", h=H),
                            in0=psums[b][:, 0:D].rearrange("p (e h) -> p e h", h=H),
                            in1=zr[:, i, :].rearrange("p (o h) -> p o h", o=1)
                                .to_broadcast([128, HD, H]),
                            op=mybir.AluOpType.mult)
                    for i, b in enumerate(blocks):
                        tpo = tp.tile([128, 192], F32, tag="tpo", name=f"tpo{i}")
                        pt = tpo[0:D, 0:128]
                        po = tpo[:, 128:192]
                        nc.tensor.transpose(out=pt, in_=vals[:, i, :],
                                            identity=idn_t[:])
                        vT = ep.tile([D, 128], BF16, tag="vT", name=f"vT{i}")
                        nc.vector.tensor_copy(vT[:], pt)
                        nc.tensor.matmul(out=po, lhsT=vT[:], rhs=wot_t[:],
                                         start=True, stop=True)
                        nc.scalar.copy(poall[:, b, :], po)

            # ---- phase C: batched LayerNorm + residual, two half-batches.
            # xpb/out use the p-major layout: DRAM row p*NBLK + a holds the
            # data for permuted node a*128 + p (1 DMA descriptor/partition).
            with tc.tile_pool(name="fin", bufs=2) as fp:
                for b0, b1 in ((0, NBLK // 2), (NBLK // 2, NBLK)):
                    nb = b1 - b0
                    pslab = poall[:, b0:b1, :]
                    nmu = fp.tile([128, NBLK // 2 + 1], F32, tag="nmu")
                    nc.vector.tensor_reduce(
                        out=nmu[:, 0:nb], in_=pslab,
                        axis=mybir.AxisListType.X, op=mybir.AluOpType.add)
                    nc.vector.tensor_scalar_mul(
                        nmu[:, 0:nb], nmu[:, 0:nb], -1.0 / D)
                    ct = fp.tile([128, NBLK // 2 + 1, D], F32, tag="ct")
                    nc.vector.tensor_tensor(
                        out=ct[:, 0:nb, :], in0=pslab,
                        in1=nmu[:, 0:nb].rearrange("p (b o) -> p b o", o=1)
                            .to_broadcast([128, nb, D]),
                        op=mybir.AluOpType.add)
                    nc.gpsimd.tensor_tensor(
                        out=ct[:, 0:nb, :], in0=ct[:, 0:nb, :],
                        in1=boc_t[:].rearrange("p (o d) -> p o d", o=1)
                            .to_broadcast([128, nb, D]),
                        op=mybir.AluOpType.add)
                    sq = fp.tile([128, NBLK // 2 + 1, D], F32, tag="sq")
                    nc.gpsimd.tensor_tensor(
                        out=sq[:, 0:nb, :], in0=ct[:, 0:nb, :],
                        in1=ct[:, 0:nb, :], op=mybir.AluOpType.mult)
                    v1 = fp.tile([128, NBLK // 2 + 1], F32, tag="v1")
                    nc.vector.tensor_reduce(
                        out=v1[:, 0:nb], in_=sq[:, 0:nb, :],
                        axis=mybir.AxisListType.X, op=mybir.AluOpType.add)
                    nc.vector.tensor_scalar(
                        out=v1[:, 0:nb], in0=v1[:, 0:nb],
                        scalar1=1.0 / D, scalar2=LN_EPS,
                        op0=mybir.AluOpType.mult, op1=mybir.AluOpType.add)
                    # rstd = exp(-0.5*ln(var+eps)): stays in the exp/ln table
                    lnv = fp.tile([128, NBLK // 2 + 1], F32, tag="lnv")
                    nc.scalar.activation(
                        out=lnv[:, 0:nb], in_=v1[:, 0:nb],
                        func=mybir.ActivationFunctionType.Ln)
                    rstd = fp.tile([128, NBLK // 2 + 1], F32, tag="rstd")
                    nc.scalar.activation(
                        out=rstd[:, 0:nb], in_=lnv[:, 0:nb], scale=-0.5,
                        func=mybir.ActivationFunctionType.Exp)
                    xb = fp.tile([128, NBLK // 2 + 1, D], F32, tag="xb")
                    nc.sync.dma_start(
                        out=xb[:, 0:nb, :],
                        in_=xpb[:].rearrange("(p a) d -> p a d", p=128)
                            [:, b0:b1, :])
                    ot = fp.tile([128, NBLK // 2 + 1, D], F32, tag="ot")
                    nc.vector.tensor_tensor(
                        out=ot[:, 0:nb, :], in0=ct[:, 0:nb, :],
                        in1=rstd[:, 0:nb].rearrange("p (b o) -> p b o", o=1)
                            .to_broadcast([128, nb, D]),
                        op=mybir.AluOpType.mult)
                    nc.gpsimd.tensor_tensor(
                        out=ot[:, 0:nb, :], in0=ot[:, 0:nb, :],
                        in1=gam_t[:].rearrange("p (o d) -> p o d", o=1)
                            .to_broadcast([128, nb, D]),
                        op=mybir.AluOpType.mult)
                    nc.gpsimd.tensor_tensor(
                        out=ot[:, 0:nb, :], in0=ot[:, 0:nb, :],
                        in1=xb[:, 0:nb, :], op=mybir.AluOpType.add)
                    nc.sync.dma_start(
                        out=out[:].rearrange("(p a) d -> p a d", p=128)
                            [:, b0:b1, :],
                        in_=ot[:, 0:nb, :])
    return nc


def kernel(x, edge_index, edge_weights, Wq, bq, Wk, bk, Wv, bv, Wo, bo,
           gamma, beta):
    x = np.asarray(x, np.float32)
    edge_index = np.asarray(edge_index)
    edge_weights = np.asarray(edge_weights, np.float32)
    origins = np.asarray(edge_index[0], np.int64)
    dests = np.asarray(edge_index[1], np.int64)

    struct, per_core = _build_structure(origins, dests, edge_weights)
    nc = _build_graph(struct)
    nc.finalize()

    # shared (replicated) host arrays.
    # v uses (e, h)-interleaved layout; k and q use it too so the d-axis
    # tree reduce groups by head with h innermost.
    vperm = (np.arange(H)[None, :] * HD + np.arange(HD)[:, None]).ravel()
    Wkf = np.asarray(Wk, np.float32)
    Wvf = np.asarray(Wv, np.float32)
    Wqf = np.asarray(Wq, np.float32)
    bkf = np.asarray(bk, np.float32)
    bvf = np.asarray(bv, np.float32)
    bqf = np.asarray(bq, np.float32)
    wkv = np.zeros((D + 1, 2 * D), np.float32)
    wkv[:D, :D] = Wkf.T[:, vperm]
    wkv[:D, D:] = Wvf.T[:, vperm]
    wkv = wkv.astype(BF16_NP)          # k/v biases fold into q.bk / boc
    # q gets H extra columns projecting x onto sum_d Wq[d,:]*bk[d] per head
    # (score = q.k_nobias + q.bk, and q.bk = x @ wqx_h + bq.bk_h)
    wq_h = np.zeros((D + 1, 2 * D), np.float32)
    wq_h[:D, 0:D] = Wqf.T[:, vperm]
    wq_h[D, 0:D] = bqf[vperm]
    for h in range(H):
        dims = np.arange(HD) + h * HD          # original k dims of head h
        wq_h[:D, D + h] = Wqf.T[:, dims] @ bkf[dims]
        wq_h[D, D + h] = bqf[dims] @ bkf[dims]
    wq_h = wq_h.astype(BF16_NP)
    wot = np.ascontiguousarray(np.asarray(Wo, np.float32).T[vperm, :])
    # bv contributes bv @ Wo.T to every output row (sum of attn = 1)
    bo2 = np.asarray(bo, np.float32) + bvf @ np.asarray(Wo, np.float32).T
    boc = np.tile((bo2 - bo2.mean())[None, :], (128, 1)).astype(np.float32)
    gam_t = np.tile(np.asarray(gamma, np.float32)[None, :], (128, 1))
    THALF = (struct["TMAX"] + 1) // 2
    iot = np.tile(np.arange(128, dtype=np.float32)[None, :, None],
                  (128, 1, THALF)).astype(BF16_NP)
    idn = np.eye(128, dtype=np.float32)

    scale = HD ** -0.5
    in_maps = []
    for c in range(NCORES):
        core = per_core[c]
        meta = _per_core_arrays(struct, core, scale)
        perm = core["perm"]
        xc = x[c * NOWN:(c + 1) * NOWN]
        xp = np.zeros((NB, D), np.float32)
        valid = perm >= 0
        xp[valid] = xc[perm[valid]]
        xTo = np.zeros((D + 1, NB), np.float32)
        xTo[:D] = xp.T
        xTo[D] = 1.0
        xTo = xTo.astype(BF16_NP)
        # kv x, dest-compacted and half-packed by table-row parity; chunk
        # ch's table rows are ranks [bounds[ch], bounds[ch+1])
        CHR = struct["chrows"]
        ucol = core["ucol"]
        bounds = core["bounds"]
        tabx = np.zeros((CH * CHR, D), np.float32)
        for ch in range(CH):
            n = int(bounds[ch + 1] - bounds[ch])
            tabx[ch * CHR:ch * CHR + n] = x[ucol[bounds[ch]:bounds[ch + 1]]]
        xTc = np.zeros((D + 1, CH * CHR), np.float32)
        xTc[:D] = tabx.T
        xTc[D] = 1.0
        xTc = xTc.astype(BF16_NP)
        # p-major: row p*NBLK + a  <-  permuted node a*128 + p
        xpb = np.ascontiguousarray(
            (xp + np.asarray(beta, np.float32)[None, :])
            .reshape(NBLK, 128, D).transpose(1, 0, 2)).reshape(NB, D)
        in_maps.append({
            "xT": xTc, "xTo": xTo, "wkv": wkv,
            "wq": wq_h, "wot": wot,
            "boc": boc, "gam": gam_t, "iot": iot, "idn": idn,
            "xpb": xpb, "meta": meta,
        })

    global LAST_SIM_NS
    if SIMULATE_COST:
        from concourse import bass_interp
        sim = bass_interp.CoreSim(nc, no_exec=True, publish_trace=False)
        sim.event_loop()
        LAST_SIM_NS = int(sim.time)

    res = run_bass_kernel_spmd(nc, in_maps, core_ids=list(range(NCORES)),
                               trace=TRACE)
    global LAST_RESULT
    LAST_RESULT = res
    full = np.zeros((N, D), np.float32)
    for c in range(NCORES):
        o = np.asarray(res.results[c]["out"])
        # p-major: row p*NBLK + a holds permuted node a*128 + p
        o = o.reshape(128, NBLK, D).transpose(1, 0, 2).reshape(NB, D)
        perm = per_core[c]["perm"]
        valid = perm >= 0
        full[c * NOWN + perm[valid]] = o[valid]
    return full


TRACE = False
SIMULATE_COST = False
LAST_RESULT = None
LAST_SIM_NS = None


# revision 64
# speedup vs baseline: 1.3653x; 1.0925x over previous
"""Trainium2 Bass kernel for BaseDependentAttentionLayer (GNN message passing).

Strategy (8 NeuronCores, SPMD, no collectives):
  - Nodes sharded by origin: core c owns nodes [c*12500, (c+1)*12500).
  - Origins are permuted into 98 blocks of <=128 per core, bin-packed so
    each (dest-chunk, block) cell holds ~<=512 edges -> near-zero tile pad.
  - Each core computes the bf16 [k|v] row table for ALL nodes (4 chunk
    tensors in DRAM so edge gathers overlap the build) and q for its own
    nodes (kept in SBUF).
  - Edges bucketed by (dest-chunk, origin-block), padded to 128-edge tiles;
    per-edge k|v fetched with dma_gather (int16 idx, 256B rows).
  - Per-slot q is NOT gathered: a pair-one-hot table (129x129 rows of
    [onehot(a)|onehot(b)], 512B) is gathered transpose-style to give
    selT[origin, slot]; one matmul per tile against the q block broadcasts
    q to slots (PSUM), copied to SBUF bf16 by ACT.
  - Segment softmax runs without max-subtraction (shift-invariant; values
    bounded); segment sums via 0/1 selection-matrix matmuls into per-block
    PSUM accumulators; sel is built by DVE is_equal at 2 elem/cycle.
  - Scores reduce via a bf16 tree-fold (2x DVE rate). exp writes directly
    into the ctb tail. LayerNorm+residual is deferred to one batched final
    phase; rstd = exp(-0.5*ln(var+eps)) so ACT never swaps tables.
"""

import sys

sys.path.insert(0, "/opt/trn_rl_repo")

import numpy as np
import ml_dtypes

import concourse.bass as bass
import concourse.bacc as bacc
import concourse.mybir as mybir
from concourse.tile import TileContext
from concourse.bass_utils import run_bass_kernel_spmd

N = 100000
E = 1600000
D = 64
H = 4
HD = 16
NCORES = 8
NOWN = 12500            # nodes owned per core
NBLK = 98               # 128-node origin blocks per core
NB = NBLK * 128         # 12544 padded own nodes
NT = 100352             # padded global table rows (= 4 * 25088)
CH = 4                  # dest chunks (int16 gather index limit)
CHROWS = NT // CH       # 25088
TCAP = 64               # max tiles per super-group (psum / sbuf budget)
BCAP = 4                # max blocks per super-group
LN_EPS = 1e-5
PAD_OID = 200.0         # origin-id sentinel for pad slots (matches no node)
POT_SYM = 129           # pair-one-hot symbols (128 origins + zero pad)

F32 = mybir.dt.float32
BF16 = mybir.dt.bfloat16
I16 = mybir.dt.int16
BF16_NP = ml_dtypes.bfloat16

SLAB = 2048             # phase-A rows per slab


def _balance_blocks(chunk_of_edge_dest, local_origin):
    """Assign local origins to NBLK blocks (<=128 each), balancing the
    per-(chunk, block) edge counts with 4-D LPT so cells pack near 512.

    Returns perm: perm[new_node_index] = local_origin (block b owns
    perm[b*128:(b+1)*128] entries; tail entries may be -1 = unused)."""
    cnt = np.zeros((NOWN, CH), np.int64)
    np.add.at(cnt, (local_origin, chunk_of_edge_dest), 1)
    deg = cnt.sum(1)
    order = np.argsort(-deg, kind="stable")
    CAP = 4 * 128            # capped blocks: every cell fits in 4 tiles
    NOVF = 2                 # overflow blocks (uncapped) at the end
    NCAPB = NBLK - NOVF
    bsum = np.zeros((NBLK, CH), np.int64)
    bcnt = np.zeros(NBLK, np.int64)
    assign = np.zeros(NOWN, np.int64)
    # heaviest origins soak into the overflow blocks so the capped blocks'
    # total fits under NCAPB*CAP per chunk
    for i, o in enumerate(order[:NOVF * 128]):
        b = NCAPB + i % NOVF
        assign[o] = b
        bsum[b] += cnt[o]
        bcnt[b] += 1
    for o in order[NOVF * 128:]:
        c = cnt[o]
        cand = (bsum[:NCAPB] + c[None, :]).max(1).astype(np.float64)
        open_ = bcnt[:NCAPB] < 128
        feas = (cand <= CAP) & open_
        if feas.any():
            # worst-fit: keep all blocks growing evenly (tight 4-D packing)
            score = np.where(feas, cand, np.inf)
        else:
            score = np.where(open_, cand, np.inf)
        b = int(np.argmin(score))
        assign[o] = b
        bsum[b] += c
        bcnt[b] += 1
    perm = np.full(NBLK * 128, -1, np.int64)
    pos = 0
    for b in range(NBLK):
        members = np.nonzero(assign == b)[0]
        perm[b * 128:b * 128 + len(members)] = members
    return perm, assign


def _build_structure(origins, dests, edge_weights):
    """Global (core-independent) tile structure + per-core slot data.

    Dest nodes are compacted per core (only referenced nodes get kv-table
    rows); chunking is by compacted rank."""
    owner = origins // NOWN
    # pass 1: per-core unique dests -> table size (shared across cores)
    ucols = []
    for c in range(NCORES):
        ucols.append(np.unique(dests[owner == c]))
    max_u = max(len(u) for u in ucols)
    chrows = -(-max_u // (4 * SLAB)) * SLAB      # per-chunk rows, 2048-mult
    assert chrows <= 32000
    per_core = []
    cell_cnt = np.zeros((NCORES, CH * NBLK), np.int64)
    for c in range(NCORES):
        m = owner == c
        o = (origins[m] - c * NOWN).astype(np.int64)
        d = dests[m].astype(np.int64)
        ew = edge_weights[m]
        rank = np.searchsorted(ucols[c], d)
        # chunk boundaries equalize EDGES per chunk (cells pack to ~512);
        # each chunk's rank span must still fit the CHR-row table
        rdeg = np.bincount(rank, minlength=len(ucols[c]))
        cum = np.cumsum(rdeg)
        nb_ = [0] + [int(np.searchsorted(cum, cum[-1] * k // CH))
                     for k in (1, 2, 3)] + [len(ucols[c])]
        bounds = np.asarray(nb_, np.int64)
        assert (np.diff(bounds) <= chrows).all()
        chunk = np.searchsorted(bounds[1:-1], rank, side="right")
        perm, assign = _balance_blocks(chunk, o)
        # origin -> (block, slot-in-block) position
        opos = np.zeros(NOWN, np.int64)
        valid = perm >= 0
        opos[perm[valid]] = np.nonzero(valid)[0]
        p = opos[o]                       # position in permuted node space
        blk = p >> 7
        cell = chunk * NBLK + blk
        order = np.argsort(cell, kind="stable")
        cnt = np.bincount(cell, minlength=CH * NBLK)
        cell_cnt[c] = cnt
        per_core.append({
            "perm": perm, "cell": cell[order], "oid": (p & 127)[order],
            "dloc": (rank - bounds[chunk])[order], "ew": ew[order],
            "cnt": cnt, "ucol": ucols[c], "bounds": bounds,
        })
    cmax = cell_cnt.max(0)
    cell_tiles = np.maximum((cmax + 127) // 128, 1)   # [CH*NBLK]

    # super-groups: consecutive blocks, <=BCAP blocks, <=TCAP tiles, T even
    blk_tiles = cell_tiles.reshape(CH, NBLK).sum(0)   # tiles per block
    sgs = []
    start = 0
    while start < NBLK:
        nb, t = 0, 0
        while (start + nb < NBLK and nb < BCAP
               and t + blk_tiles[start + nb] <= TCAP):
            t += blk_tiles[start + nb]
            nb += 1
        if nb == 0:
            nb, t = 1, int(blk_tiles[start])
        blocks = list(range(start, start + nb))
        pad_tile = t % 2                # keep T even for the selT gather
        sgs.append({"blocks": blocks, "T": t + pad_tile, "pad_tile": pad_tile})
        start += nb

    # stream order: sg -> chunk -> block; the optional pad tile sits at the
    # end of the last chunk segment and belongs to the sg's first block.
    t_off = 0
    cell_tile_off = np.zeros(CH * NBLK, np.int64)
    for sg in sgs:
        sg["tile_off"] = t_off
        T_c = []
        tile_blocks = []
        for ch in range(CH):
            tc = 0
            for b in sg["blocks"]:
                cell = ch * NBLK + b
                nt = int(cell_tiles[cell])
                cell_tile_off[cell] = t_off
                tile_blocks.extend([b] * nt)
                t_off += nt
                tc += nt
            if ch == CH - 1 and sg["pad_tile"]:
                tile_blocks.append(sg["blocks"][0])
                t_off += 1
                tc += 1
            T_c.append(tc)
        first, last = {}, {}
        for i, b in enumerate(tile_blocks):
            if b not in first:
                first[b] = i
            last[b] = i
        sg["T_c"] = T_c
        sg["tile_blocks"] = tile_blocks
        sg["first"] = first
        sg["last"] = last
    S_tiles = t_off
    struct = {"sgs": sgs, "S_tiles": S_tiles, "cell_tiles": cell_tiles,
              "cell_tile_off": cell_tile_off, "chrows": int(chrows),
              "TMAX": max(sg["T"] for sg in sgs)}
    return struct, per_core


META_W = 21             # int16 units per tile: oid 1 + ew 4 + kvx 8 + qx 8


def _per_core_arrays(struct, core, scale):
    """Packed per-core metadata [128, S_tiles*META_W] int16 plus host perm."""
    S_tiles = struct["S_tiles"]
    S = S_tiles * 128
    cell_tile_off = struct["cell_tile_off"]

    oid = np.full(S, PAD_OID, np.float32)
    kvi = np.zeros(S, np.int16)
    ew4 = np.zeros((S, H), np.float32)
    qxi = np.zeros(S, np.int16)

    cnt = core["cnt"]
    cell_edge_off = np.zeros(CH * NBLK + 1, np.int64)
    np.cumsum(cnt, out=cell_edge_off[1:])
    for cell in range(CH * NBLK):
        n = int(cnt[cell])
        if n == 0:
            continue
        e0 = cell_edge_off[cell]
        s0 = cell_tile_off[cell] * 128
        sl = slice(s0, s0 + n)
        el = slice(e0, e0 + n)
        oid[sl] = core["oid"][el].astype(np.float32)
        kvi[sl] = core["dloc"][el].astype(np.int16)
        ew4[sl] = core["ew"][el] * scale
        qxi[sl] = (core["oid"][el] * NBLK + (cell % NBLK)).astype(np.int16)

    def wrap(run_vals):
        w = run_vals.reshape(-1, 16).T
        return np.tile(w, (8, 1))

    # per-sg contiguous regions: [oid(T) | ew(T*4) | kvx(T*8) | prx(T*4)]
    oid_pt = np.ascontiguousarray(
        oid.reshape(S_tiles, 128).T).astype(BF16_NP).view(np.int16)
    ew_pt = ew4.astype(BF16_NP).view(np.int16) \
        .reshape(S_tiles, 128, H).transpose(1, 0, 2)
    meta = np.zeros((128, S_tiles * META_W), np.int16)
    for sg in struct["sgs"]:
        t0, T = sg["tile_off"], sg["T"]
        m0 = t0 * META_W
        meta[:, m0:m0 + T] = oid_pt[:, t0:t0 + T]
        meta[:, m0 + T:m0 + 5 * T] = \
            ew_pt[:, t0:t0 + T, :].reshape(128, T * H)
        meta[:, m0 + 5 * T:m0 + 13 * T] = \
            wrap(kvi[t0 * 128:(t0 + T) * 128])
        meta[:, m0 + 13 * T:m0 + 21 * T] = \
            wrap(qxi[t0 * 128:(t0 + T) * 128])
    return meta


def _build_graph(struct):
    nc = bacc.Bacc()
    S_tiles = struct["S_tiles"]
    TMAX = struct["TMAX"]
    sgs = struct["sgs"]
    CHR = struct["chrows"]

    # x for the kv table, dest-compacted, with a ones row for the bias
    xT = nc.declare_dram_parameter("xT", [D + 1, CH * CHR], BF16,
                                   isOutput=False)
    xTo = nc.declare_dram_parameter("xTo", [D + 1, NB], BF16, isOutput=False)
    wkv = nc.declare_dram_parameter("wkv", [D + 1, 2 * D], BF16, isOutput=False)
    wq = nc.declare_dram_parameter("wq", [D + 1, 2 * D], BF16, isOutput=False)
    wot = nc.declare_dram_parameter("wot", [D, D], F32, isOutput=False)
    boc = nc.declare_dram_parameter("boc", [128, D], F32, isOutput=False)
    gam = nc.declare_dram_parameter("gam", [128, D], F32, isOutput=False)
    THALF = (TMAX + 1) // 2
    iot = nc.declare_dram_parameter("iot", [128, 128, THALF], BF16, isOutput=False)
    idn = nc.declare_dram_parameter("idn", [128, 128], F32, isOutput=False)
    xpb = nc.declare_dram_parameter("xpb", [NB, D], F32, isOutput=False)
    meta = nc.declare_dram_parameter("meta", [128, S_tiles * META_W], I16,
                                     isOutput=False)
    out = nc.declare_dram_parameter("out", [NB, D], F32, isOutput=True)

    kv_tab = [nc.dram_tensor(f"kv_tab{ch}", [CHR, 2 * D], BF16)
              for ch in range(CH)]
    q_tab = nc.dram_tensor("q_tab", [NB, 2 * D], BF16)

    with TileContext(nc) as tc:
        with tc.tile_pool(name="const", bufs=1) as cp:
            wkv_t = cp.tile([D + 1, 2 * D], BF16)
            nc.sync.dma_start(out=wkv_t[:], in_=wkv[:])
            wq_t = cp.tile([D + 1, 2 * D], BF16)
            nc.sync.dma_start(out=wq_t[:], in_=wq[:])
            wot_f = cp.tile([D, D], F32)
            nc.sync.dma_start(out=wot_f[:], in_=wot[:])
            wot_t = cp.tile([D, D], BF16)
            nc.vector.tensor_copy(wot_t[:], wot_f[:])
            boc_t = cp.tile([128, D], F32)
            nc.sync.dma_start(out=boc_t[:], in_=boc[:])
            gam_t = cp.tile([128, D], F32)
            nc.sync.dma_start(out=gam_t[:], in_=gam[:])
            iot_t = cp.tile([128, 128, THALF], BF16)
            nc.sync.dma_start(out=iot_t[:], in_=iot[:])
            idn_t = cp.tile([128, 128], F32)
            nc.sync.dma_start(out=idn_t[:], in_=idn[:])
            poall = cp.tile([128, NBLK, D], F32)       # post-Wo, pre-LN

            # meta prefetch: first 6 sgs' metadata loads issue before the
            # phase-A DMAs queue up on SP, so edge gathers start early
            sg_order = sorted(sgs, key=lambda g: -g["T"])
            mp_cm = tc.tile_pool(name="met", bufs=6)
            mp = mp_cm.__enter__()
            mts = {}

            def load_meta(i):
                g = sg_order[i]
                m = mp.tile([128, TMAX * META_W], I16, tag="mt",
                            name=f"mt{i}")
                nc.sync.dma_start(
                    out=m[:, 0:g["T"] * META_W],
                    in_=meta[:, g["tile_off"] * META_W:
                             (g["tile_off"] + g["T"]) * META_W])
                mts[i] = m

            for i in range(min(6, len(sg_order))):
                load_meta(i)

            # ---- phase A: q (SBUF) first, then kv chunk tables (DRAM).
            # x comes half-packed [130, rows/2]: even rows in partitions
            # 0..64, odd rows in 65..129 (halves the DMA column count).
            BIG = 8192
            with (
                tc.tile_pool(name="pa", bufs=3) as pa,
                tc.tile_pool(name="paq", bufs=1) as paq,
                tc.tile_pool(name="pap", bufs=2, space="PSUM") as pap,
            ):
                # q: one load; block b = columns [b*128, (b+1)*128)
                xq = paq.tile([D + 1, NB], BF16)
                nc.sync.dma_start(out=xq[:], in_=xTo[:])
                QB = 16                      # blocks per psum slab
                for s in range((NBLK + QB - 1) // QB):
                    b0 = s * QB
                    nblk = min(QB, NBLK - b0)
                    ps = pap.tile([128, (SLAB // 128) * 2 * D], F32, tag="ps",
                                  name=f"psq{s}")
                    psv = ps[:].rearrange("p (a d) -> p a d", d=128)
                    for j in range(nblk):
                        b = b0 + j
                        nc.tensor.matmul(
                            out=psv[:, j, :],
                            lhsT=xq[:, b * 128:(b + 1) * 128],
                            rhs=wq_t[:],
                            start=True, stop=True)
                    qb = pa.tile([128, SLAB // 128, 2 * D], BF16, tag="qb")
                    nc.vector.tensor_copy(
                        qb[:, 0:nblk, :].rearrange("p a d -> p (a d)"),
                        ps[:, 0:nblk * 2 * D])
                    # q_tab row p*NBLK + b (p-major: contiguous per partition)
                    nc.scalar.dma_start(
                        out=q_tab[:].rearrange("(p a) d -> p a d", p=128)
                            [:, b0:b0 + nblk, :],
                        in_=qb[:, 0:nblk, :])

                # kv: 8192-row DMA slabs, 2048-row GEMM sub-slabs. Partition
                # p holds table rows [n0+rpp8*p, +rpp8); tile j of sub-slab
                # s2 covers rows {rpp8*p + spp*s2 + j} (p-strided lhsT).
                for ch in range(CH):
                    n0 = 0
                    while n0 < CHR:
                        nrows = min(BIG, CHR - n0)
                        rpp8 = nrows // 128
                        xs = pa.tile([D + 1, BIG], BF16, tag="xs")
                        nc.sync.dma_start(
                            out=xs[:, 0:nrows],
                            in_=xT[:, ch * CHR + n0:ch * CHR + n0 + nrows])
                        sb = pa.tile([128, BIG // 128, 2 * D], BF16, tag="sb")
                        nsub = nrows // SLAB
                        spp = rpp8 // nsub       # rows per partition per sub
                        for s2 in range(nsub):
                            ps = pap.tile([128, (SLAB // 128) * 2 * D], F32,
                                          tag="ps", name=f"pkv{ch}_{n0}_{s2}")
                            for j in range(SLAB // 128):
                                r = spp * s2 + j
                                nc.tensor.matmul(
                                    out=ps[:, j * 2 * D:(j + 1) * 2 * D],
                                    lhsT=xs[:, r:nrows:rpp8],
                                    rhs=wkv_t[:],
                                    start=True, stop=True)
                            dst = sb[:, spp * s2:spp * (s2 + 1), :] \
                                .rearrange("p a d -> p (a d)")
                            nc.vector.tensor_copy(
                                dst, ps[:, 0:SLAB // 128 * 2 * D])
                        nc.scalar.dma_start(
                            out=kv_tab[ch][n0:n0 + nrows, :]
                                .rearrange("(p a) d -> p a d", p=128),
                            in_=sb[:, 0:rpp8, :])
                        n0 += nrows

            # ---- phase B: edge processing per super-group
            with (
                tc.tile_pool(name="gat", bufs=2) as gp,
                tc.tile_pool(name="wrk", bufs=2) as wp,
                tc.tile_pool(name="dvi", bufs=1) as dv,
                tc.tile_pool(name="eps", bufs=2) as ep,
                tc.tile_pool(name="bps", bufs=6, space="PSUM") as bp,
                tc.tile_pool(name="tpo", bufs=2, space="PSUM") as tp,
            ):
                for sgi, sg in enumerate(sg_order):
                    T = sg["T"]
                    t0 = sg["tile_off"]
                    mt = mts.pop(sgi)
                    if sgi + 6 < len(sg_order):
                        load_meta(sgi + 6)
                    oid_t = mt[:, 0:T].bitcast(BF16)
                    ew_t = mt[:, T:5 * T].bitcast(BF16) \
                        .rearrange("p (t w) -> p t w", w=H)
                    kvi_t = mt[:, 5 * T:13 * T]
                    qxi_t = mt[:, 13 * T:21 * T]

                    kvg = gp.tile([128, TMAX, 2 * D], BF16, tag="kvg")
                    off = 0
                    for ch in range(CH):
                        tcn = sg["T_c"][ch]
                        if tcn == 0:
                            continue
                        nc.gpsimd.dma_gather(
                            out_ap=kvg[:, off:off + tcn, :],
                            in_ap=kv_tab[ch][:],
                            idxs_ap=kvi_t[:, off * 8:(off + tcn) * 8],
                            num_idxs=tcn * 128,
                            num_idxs_reg=tcn * 128,
                            elem_size=2 * D,
                            single_packet=False)
                        off += tcn
                    qg = gp.tile([128, TMAX, 2 * D], BF16, tag="qg")
                    nc.gpsimd.dma_gather(
                        out_ap=qg[:, 0:T, :],
                        in_ap=q_tab[:],
                        idxs_ap=qxi_t[:],
                        num_idxs=T * 128,
                        num_idxs_reg=T * 128,
                        elem_size=2 * D,
                        single_packet=False)

                    # scores: qk mult (2x bf16) then bf16 tree reduce over d
                    qk = dv.tile([128, TMAX, HD, H], BF16, tag="qk")
                    nc.vector.tensor_tensor(
                        out=qk[:, 0:T, :, :],
                        in0=qg[:, 0:T, 0:D].rearrange("p t (e h) -> p t e h", h=H),
                        in1=kvg[:, 0:T, 0:D].rearrange("p t (e h) -> p t e h", h=H),
                        op=mybir.AluOpType.mult)
                    r8 = dv.tile([128, TMAX, 8, H], BF16, tag="r8")
                    nc.vector.tensor_tensor(
                        out=r8[:, 0:T, :, :], in0=qk[:, 0:T, 0:8, :],
                        in1=qk[:, 0:T, 8:16, :], op=mybir.AluOpType.add)
                    r4 = dv.tile([128, TMAX, 4, H], BF16, tag="r4")
                    nc.vector.tensor_tensor(
                        out=r4[:, 0:T, :, :], in0=r8[:, 0:T, 0:4, :],
                        in1=r8[:, 0:T, 4:8, :], op=mybir.AluOpType.add)
                    r2 = dv.tile([128, TMAX, 2, H], BF16, tag="r2")
                    nc.vector.tensor_tensor(
                        out=r2[:, 0:T, :, :], in0=r4[:, 0:T, 0:2, :],
                        in1=r4[:, 0:T, 2:4, :], op=mybir.AluOpType.add)
                    ws = dv.tile([128, TMAX, 1, H], BF16, tag="ws")
                    nc.vector.tensor_tensor(
                        out=ws[:, 0:T, :, :], in0=r2[:, 0:T, 0:1, :],
                        in1=r2[:, 0:T, 1:2, :], op=mybir.AluOpType.add)
                    wsb = dv.tile([128, TMAX, H], BF16, tag="wsb")
                    nc.vector.tensor_tensor(
                        out=wsb[:, 0:T, :], in0=ws[:, 0:T, 0, :],
                        in1=qg[:, 0:T, D:D + H], op=mybir.AluOpType.add)
                    wse = wp.tile([128, TMAX, H], BF16, tag="wse")
                    nc.vector.tensor_tensor(
                        out=wse[:, 0:T, :],
                        in0=wsb[:, 0:T, :],
                        in1=ew_t[:],
                        op=mybir.AluOpType.mult)
                    ctb = wp.tile([128, TMAX, D + H], BF16, tag="ctb")
                    nc.scalar.activation(
                        out=ctb[:, 0:T, D:D + H], in_=wse[:, 0:T, :],
                        func=mybir.ActivationFunctionType.Exp)
                    nc.vector.tensor_tensor(
                        out=ctb[:, 0:T, 0:D].rearrange("p t (e h) -> p t e h", h=H),
                        in0=kvg[:, 0:T, D:2 * D].rearrange("p t (e h) -> p t e h", h=H),
                        in1=ctb[:, 0:T, D:D + H].rearrange("p t (o h) -> p t o h", o=1)
                            .to_broadcast([128, T, HD, H]),
                        op=mybir.AluOpType.mult)

                    # sel one-hots + scatter matmuls into per-block psums
                    # (two instrs so the iota const is only TMAX/2 deep)
                    sel = wp.tile([128, 128, TMAX], BF16, tag="sel")
                    ha = T // 2
                    for s0, sn in ((0, ha), (ha, T - ha)):
                        nc.vector.tensor_tensor(
                            out=sel[:, :, s0:s0 + sn],
                            in0=oid_t[:, s0:s0 + sn]
                                .rearrange("p (o t) -> p o t", o=1)
                                .to_broadcast([128, 128, sn]),
                            in1=iot_t[:, :, 0:sn],
                            op=mybir.AluOpType.is_equal)
                    psums = {}
                    for i, b in enumerate(sg["blocks"]):
                        psums[b] = bp.tile([128, D + H], F32, tag="bps",
                                           name=f"bps{b}")[:]
                    for i, b in enumerate(sg["tile_blocks"]):
                        nc.tensor.matmul(
                            out=psums[b][:],
                            lhsT=sel[:, :, i],
                            rhs=ctb[:, i, :],
                            start=(sg["first"][b] == i),
                            stop=(sg["last"][b] == i))

                    # per-sg epilogue: divide by z, transpose, Wo -> poall
                    blocks = sg["blocks"]
                    NBk = len(blocks)
                    zr = ep.tile([128, BCAP, H], F32, tag="zr")
                    for i, b in enumerate(blocks):
                        nc.vector.tensor_scalar_add(
                            zr[:, i, :], psums[b][:, D:D + H], 1e-16)
                    nc.vector.reciprocal(zr[:, 0:NBk, :], zr[:, 0:NBk, :])
                    vals = ep.tile([128, BCAP, D], F32, tag="vals")
                    for i, b in enumerate(blocks):
                        nc.vector.tensor_tensor(
                            out=vals[:, i, :].rearrange("p (e h) -> p e h", h=H),
                            in0=psums[b][:, 0:D].rearrange("p (e h) -> p e h", h=H),
                            in1=zr[:, i, :].rearrange("p (o h) -> p o h", o=1)
                                .to_broadcast([128, HD, H]),
                            op=mybir.AluOpType.mult)
                    for i, b in enumerate(blocks):
                        tpo = tp.tile([128, 192], F32, tag="tpo", name=f"tpo{i}")
                        pt = tpo[0:D, 0:128]
                        po = tpo[:, 128:192]
                        nc.tensor.transpose(out=pt, in_=vals[:, i, :],
                                            identity=idn_t[:])
                        vT = ep.tile([D, 128], BF16, tag="vT", name=f"vT{i}")
                        nc.scalar.copy(vT[:], pt)
                        nc.tensor.matmul(out=po, lhsT=vT[:], rhs=wot_t[:],
                                         start=True, stop=True)
                        nc.scalar.copy(poall[:, b, :], po)

            mp_cm.__exit__(None, None, None)

            # ---- phase C: batched LayerNorm + residual, two half-batches.
            # xpb/out use the p-major layout: DRAM row p*NBLK + a holds the
            # data for permuted node a*128 + p (1 DMA descriptor/partition).
            with tc.tile_pool(name="fin", bufs=2) as fp:
                for b0, b1 in ((0, NBLK // 2), (NBLK // 2, NBLK)):
                    nb = b1 - b0
                    pslab = poall[:, b0:b1, :]
                    nmu = fp.tile([128, NBLK // 2 + 1], F32, tag="nmu")
                    nc.vector.tensor_reduce(
                        out=nmu[:, 0:nb], in_=pslab,
                        axis=mybir.AxisListType.X, op=mybir.AluOpType.add)
                    nc.vector.tensor_scalar_mul(
                        nmu[:, 0:nb], nmu[:, 0:nb], -1.0 / D)
                    ct = fp.tile([128, NBLK // 2 + 1, D], F32, tag="ct")
                    nc.vector.tensor_tensor(
                        out=ct[:, 0:nb, :], in0=pslab,
                        in1=nmu[:, 0:nb].rearrange("p (b o) -> p b o", o=1)
                            .to_broadcast([128, nb, D]),
                        op=mybir.AluOpType.add)
                    nc.gpsimd.tensor_tensor(
                        out=ct[:, 0:nb, :], in0=ct[:, 0:nb, :],
                        in1=boc_t[:].rearrange("p (o d) -> p o d", o=1)
                            .to_broadcast([128, nb, D]),
                        op=mybir.AluOpType.add)
                    sq = fp.tile([128, NBLK // 2 + 1, D], F32, tag="sq")
                    nc.gpsimd.tensor_tensor(
                        out=sq[:, 0:nb, :], in0=ct[:, 0:nb, :],
                        in1=ct[:, 0:nb, :], op=mybir.AluOpType.mult)
                    v1 = fp.tile([128, NBLK // 2 + 1], F32, tag="v1")
                    nc.vector.tensor_reduce(
                        out=v1[:, 0:nb], in_=sq[:, 0:nb, :],
                        axis=mybir.AxisListType.X, op=mybir.AluOpType.add)
                    nc.vector.tensor_scalar(
                        out=v1[:, 0:nb], in0=v1[:, 0:nb],
                        scalar1=1.0 / D, scalar2=LN_EPS,
                        op0=mybir.AluOpType.mult, op1=mybir.AluOpType.add)
                    # rstd = exp(-0.5*ln(var+eps)): stays in the exp/ln table
                    lnv = fp.tile([128, NBLK // 2 + 1], F32, tag="lnv")
                    nc.scalar.activation(
                        out=lnv[:, 0:nb], in_=v1[:, 0:nb],
                        func=mybir.ActivationFunctionType.Ln)
                    rstd = fp.tile([128, NBLK // 2 + 1], F32, tag="rstd")
                    nc.scalar.activation(
                        out=rstd[:, 0:nb], in_=lnv[:, 0:nb], scale=-0.5,
                        func=mybir.ActivationFunctionType.Exp)
                    xb = fp.tile([128, NBLK // 2 + 1, D], F32, tag="xb")
                    nc.sync.dma_start(
                        out=xb[:, 0:nb, :],
                        in_=xpb[:].rearrange("(p a) d -> p a d", p=128)
                            [:, b0:b1, :])
                    ot = fp.tile([128, NBLK // 2 + 1, D], F32, tag="ot")
                    nc.vector.tensor_tensor(
                        out=ot[:, 0:nb, :], in0=ct[:, 0:nb, :],
                        in1=rstd[:, 0:nb].rearrange("p (b o) -> p b o", o=1)
                            .to_broadcast([128, nb, D]),
                        op=mybir.AluOpType.mult)
                    nc.gpsimd.tensor_tensor(
                        out=ot[:, 0:nb, :], in0=ot[:, 0:nb, :],
                        in1=gam_t[:].rearrange("p (o d) -> p o d", o=1)
                            .to_broadcast([128, nb, D]),
                        op=mybir.AluOpType.mult)
                    nc.gpsimd.tensor_tensor(
                        out=ot[:, 0:nb, :], in0=ot[:, 0:nb, :],
                        in1=xb[:, 0:nb, :], op=mybir.AluOpType.add)
                    nc.sync.dma_start(
                        out=out[:].rearrange("(p a) d -> p a d", p=128)
                            [:, b0:b1, :],
                        in_=ot[:, 0:nb, :])
    return nc


def kernel(x, edge_index, edge_weights, Wq, bq, Wk, bk, Wv, bv, Wo, bo,
           gamma, beta):
    x = np.asarray(x, np.float32)
    edge_index = np.asarray(edge_index)
    edge_weights = np.asarray(edge_weights, np.float32)
    origins = np.asarray(edge_index[0], np.int64)
    dests = np.asarray(edge_index[1], np.int64)

    struct, per_core = _build_structure(origins, dests, edge_weights)
    nc = _build_graph(struct)
    nc.finalize()

    # shared (replicated) host arrays.
    # v uses (e, h)-interleaved layout; k and q use it too so the d-axis
    # tree reduce groups by head with h innermost.
    vperm = (np.arange(H)[None, :] * HD + np.arange(HD)[:, None]).ravel()
    Wkf = np.asarray(Wk, np.float32)
    Wvf = np.asarray(Wv, np.float32)
    Wqf = np.asarray(Wq, np.float32)
    bkf = np.asarray(bk, np.float32)
    bvf = np.asarray(bv, np.float32)
    bqf = np.asarray(bq, np.float32)
    wkv = np.zeros((D + 1, 2 * D), np.float32)
    wkv[:D, :D] = Wkf.T[:, vperm]
    wkv[:D, D:] = Wvf.T[:, vperm]
    wkv = wkv.astype(BF16_NP)          # k/v biases fold into q.bk / boc
    # q gets H extra columns projecting x onto sum_d Wq[d,:]*bk[d] per head
    # (score = q.k_nobias + q.bk, and q.bk = x @ wqx_h + bq.bk_h)
    wq_h = np.zeros((D + 1, 2 * D), np.float32)
    wq_h[:D, 0:D] = Wqf.T[:, vperm]
    wq_h[D, 0:D] = bqf[vperm]
    for h in range(H):
        dims = np.arange(HD) + h * HD          # original k dims of head h
        wq_h[:D, D + h] = Wqf.T[:, dims] @ bkf[dims]
        wq_h[D, D + h] = bqf[dims] @ bkf[dims]
    wq_h = wq_h.astype(BF16_NP)
    wot = np.ascontiguousarray(np.asarray(Wo, np.float32).T[vperm, :])
    # bv contributes bv @ Wo.T to every output row (sum of attn = 1)
    bo2 = np.asarray(bo, np.float32) + bvf @ np.asarray(Wo, np.float32).T
    boc = np.tile((bo2 - bo2.mean())[None, :], (128, 1)).astype(np.float32)
    gam_t = np.tile(np.asarray(gamma, np.float32)[None, :], (128, 1))
    THALF = (struct["TMAX"] + 1) // 2
    iot = np.tile(np.arange(128, dtype=np.float32)[None, :, None],
                  (128, 1, THALF)).astype(BF16_NP)
    idn = np.eye(128, dtype=np.float32)

    scale = HD ** -0.5
    in_maps = []
    for c in range(NCORES):
        core = per_core[c]
        meta = _per_core_arrays(struct, core, scale)
        perm = core["perm"]
        xc = x[c * NOWN:(c + 1) * NOWN]
        xp = np.zeros((NB, D), np.float32)
        valid = perm >= 0
        xp[valid] = xc[perm[valid]]
        xTo = np.zeros((D + 1, NB), np.float32)
        xTo[:D] = xp.T
        xTo[D] = 1.0
        xTo = xTo.astype(BF16_NP)
        # kv x, dest-compacted and half-packed by table-row parity; chunk
        # ch's table rows are ranks [bounds[ch], bounds[ch+1])
        CHR = struct["chrows"]
        ucol = core["ucol"]
        bounds = core["bounds"]
        tabx = np.zeros((CH * CHR, D), np.float32)
        for ch in range(CH):
            n = int(bounds[ch + 1] - bounds[ch])
            tabx[ch * CHR:ch * CHR + n] = x[ucol[bounds[ch]:bounds[ch + 1]]]
        xTc = np.zeros((D + 1, CH * CHR), np.float32)
        xTc[:D] = tabx.T
        xTc[D] = 1.0
        xTc = xTc.astype(BF16_NP)
        # p-major: row p*NBLK + a  <-  permuted node a*128 + p
        xpb = np.ascontiguousarray(
            (xp + np.asarray(beta, np.float32)[None, :])
            .reshape(NBLK, 128, D).transpose(1, 0, 2)).reshape(NB, D)
        in_maps.append({
            "xT": xTc, "xTo": xTo, "wkv": wkv,
            "wq": wq_h, "wot": wot,
            "boc": boc, "gam": gam_t, "iot": iot, "idn": idn,
            "xpb": xpb, "meta": meta,
        })

    global LAST_SIM_NS
    if SIMULATE_COST:
        from concourse import bass_interp
        sim = bass_interp.CoreSim(nc, no_exec=True, publish_trace=False)
        sim.event_loop()
        LAST_SIM_NS = int(sim.time)

    res = run_bass_kernel_spmd(nc, in_maps, core_ids=list(range(NCORES)),
                               trace=TRACE)
    global LAST_RESULT
    LAST_RESULT = res
    full = np.zeros((N, D), np.float32)
    for c in range(NCORES):
        o = np.asarray(res.results[c]["out"])
        # p-major: row p*NBLK + a holds permuted node a*128 + p
        o = o.reshape(128, NBLK, D).transpose(1, 0, 2).reshape(NB, D)
        perm = per_core[c]["perm"]
        valid = perm >= 0
        full[c * NOWN + perm[valid]] = o[valid]
    return full


TRACE = False
SIMULATE_COST = False
LAST_RESULT = None
LAST_SIM_NS = None


# revision 65
# speedup vs baseline: 1.4361x; 1.0519x over previous
"""Trainium2 Bass kernel for BaseDependentAttentionLayer (GNN message passing).

Strategy (8 NeuronCores, SPMD, no collectives):
  - Nodes sharded by origin: core c owns nodes [c*12500, (c+1)*12500).
  - Origins are permuted into 98 blocks of <=128 per core, bin-packed so
    each (dest-chunk, block) cell holds ~<=512 edges -> near-zero tile pad.
  - Each core computes the bf16 [k|v] row table for ALL nodes (4 chunk
    tensors in DRAM so edge gathers overlap the build) and q for its own
    nodes (kept in SBUF).
  - Edges bucketed by (dest-chunk, origin-block), padded to 128-edge tiles;
    per-edge k|v fetched with dma_gather (int16 idx, 256B rows).
  - Per-slot q is NOT gathered: a pair-one-hot table (129x129 rows of
    [onehot(a)|onehot(b)], 512B) is gathered transpose-style to give
    selT[origin, slot]; one matmul per tile against the q block broadcasts
    q to slots (PSUM), copied to SBUF bf16 by ACT.
  - Segment softmax runs without max-subtraction (shift-invariant; values
    bounded); segment sums via 0/1 selection-matrix matmuls into per-block
    PSUM accumulators; sel is built by DVE is_equal at 2 elem/cycle.
  - Scores reduce via a bf16 tree-fold (2x DVE rate). exp writes directly
    into the ctb tail. LayerNorm+residual is deferred to one batched final
    phase; rstd = exp(-0.5*ln(var+eps)) so ACT never swaps tables.
"""

import sys

sys.path.insert(0, "/opt/trn_rl_repo")

import numpy as np
import ml_dtypes

import concourse.bass as bass
import concourse.bacc as bacc
import concourse.mybir as mybir
from concourse.tile import TileContext
from concourse.bass_utils import run_bass_kernel_spmd

N = 100000
E = 1600000
D = 64
H = 4
HD = 16
NCORES = 8
NOWN = 12500            # nodes owned per core
NBLK = 98               # 128-node origin blocks per core
NB = NBLK * 128         # 12544 padded own nodes
NT = 100352             # padded global table rows (= 4 * 25088)
CH = 4                  # dest chunks (int16 gather index limit)
CHROWS = NT // CH       # 25088
TCAP = 64               # max tiles per super-group (psum / sbuf budget)
BCAP = 4                # max blocks per super-group
LN_EPS = 1e-5
PAD_OID = 200.0         # origin-id sentinel for pad slots (matches no node)
POT_SYM = 129           # pair-one-hot symbols (128 origins + zero pad)

F32 = mybir.dt.float32
BF16 = mybir.dt.bfloat16
I16 = mybir.dt.int16
BF16_NP = ml_dtypes.bfloat16

SLAB = 2048             # phase-A rows per slab


def _balance_blocks(chunk_of_edge_dest, local_origin):
    """Assign local origins to NBLK blocks (<=128 each), balancing the
    per-(chunk, block) edge counts with 4-D LPT so cells pack near 512.

    Returns perm: perm[new_node_index] = local_origin (block b owns
    perm[b*128:(b+1)*128] entries; tail entries may be -1 = unused)."""
    cnt = np.zeros((NOWN, CH), np.int64)
    np.add.at(cnt, (local_origin, chunk_of_edge_dest), 1)
    deg = cnt.sum(1)
    order = np.argsort(-deg, kind="stable")
    CAP = 4 * 128            # capped blocks: every cell fits in 4 tiles
    NOVF = 2                 # overflow blocks (uncapped) at the end
    NCAPB = NBLK - NOVF
    bsum = np.zeros((NBLK, CH), np.int64)
    bcnt = np.zeros(NBLK, np.int64)
    assign = np.zeros(NOWN, np.int64)
    # heaviest origins soak into the overflow blocks so the capped blocks'
    # total fits under NCAPB*CAP per chunk
    for i, o in enumerate(order[:NOVF * 128]):
        b = NCAPB + i % NOVF
        assign[o] = b
        bsum[b] += cnt[o]
        bcnt[b] += 1
    for o in order[NOVF * 128:]:
        c = cnt[o]
        cand = (bsum[:NCAPB] + c[None, :]).max(1).astype(np.float64)
        open_ = bcnt[:NCAPB] < 128
        feas = (cand <= CAP) & open_
        if feas.any():
            # worst-fit: keep all blocks growing evenly (tight 4-D packing)
            score = np.where(feas, cand, np.inf)
        else:
            score = np.where(open_, cand, np.inf)
        b = int(np.argmin(score))
        assign[o] = b
        bsum[b] += c
        bcnt[b] += 1
    perm = np.full(NBLK * 128, -1, np.int64)
    pos = 0
    for b in range(NBLK):
        members = np.nonzero(assign == b)[0]
        perm[b * 128:b * 128 + len(members)] = members
    return perm, assign


def _build_structure(origins, dests, edge_weights):
    """Global (core-independent) tile structure + per-core slot data.

    Dest nodes are compacted per core (only referenced nodes get kv-table
    rows); chunking is by compacted rank."""
    owner = origins // NOWN
    # pass 1: per-core unique dests -> table size (shared across cores)
    ucols = []
    for c in range(NCORES):
        ucols.append(np.unique(dests[owner == c]))
    max_u = max(len(u) for u in ucols)
    chrows = -(-max_u // (4 * SLAB)) * SLAB      # per-chunk rows, 2048-mult
    assert chrows <= 32000
    per_core = []
    cell_cnt = np.zeros((NCORES, CH * NBLK), np.int64)
    for c in range(NCORES):
        m = owner == c
        o = (origins[m] - c * NOWN).astype(np.int64)
        d = dests[m].astype(np.int64)
        ew = edge_weights[m]
        rank = np.searchsorted(ucols[c], d)
        # chunk boundaries equalize EDGES per chunk (cells pack to ~512);
        # each chunk's rank span must still fit the CHR-row table
        rdeg = np.bincount(rank, minlength=len(ucols[c]))
        cum = np.cumsum(rdeg)
        nb_ = [0] + [int(np.searchsorted(cum, cum[-1] * k // CH))
                     for k in (1, 2, 3)] + [len(ucols[c])]
        bounds = np.asarray(nb_, np.int64)
        assert (np.diff(bounds) <= chrows).all()
        chunk = np.searchsorted(bounds[1:-1], rank, side="right")
        perm, assign = _balance_blocks(chunk, o)
        # origin -> (block, slot-in-block) position
        opos = np.zeros(NOWN, np.int64)
        valid = perm >= 0
        opos[perm[valid]] = np.nonzero(valid)[0]
        p = opos[o]                       # position in permuted node space
        blk = p >> 7
        cell = chunk * NBLK + blk
        order = np.argsort(cell, kind="stable")
        cnt = np.bincount(cell, minlength=CH * NBLK)
        cell_cnt[c] = cnt
        per_core.append({
            "perm": perm, "cell": cell[order], "oid": (p & 127)[order],
            "dloc": (rank - bounds[chunk])[order], "ew": ew[order],
            "cnt": cnt, "ucol": ucols[c], "bounds": bounds,
        })
    cmax = cell_cnt.max(0)
    cell_tiles = np.maximum((cmax + 127) // 128, 1)   # [CH*NBLK]

    # super-groups: consecutive blocks, <=BCAP blocks, <=TCAP tiles, T even
    blk_tiles = cell_tiles.reshape(CH, NBLK).sum(0)   # tiles per block
    sgs = []
    start = 0
    while start < NBLK:
        nb, t = 0, 0
        while (start + nb < NBLK and nb < BCAP
               and t + blk_tiles[start + nb] <= TCAP):
            t += blk_tiles[start + nb]
            nb += 1
        if nb == 0:
            nb, t = 1, int(blk_tiles[start])
        blocks = list(range(start, start + nb))
        pad_tile = t % 2                # keep T even for the selT gather
        sgs.append({"blocks": blocks, "T": t + pad_tile, "pad_tile": pad_tile})
        start += nb

    # stream order: sg -> chunk -> block; the optional pad tile sits at the
    # end of the last chunk segment and belongs to the sg's first block.
    t_off = 0
    cell_tile_off = np.zeros(CH * NBLK, np.int64)
    for sg in sgs:
        sg["tile_off"] = t_off
        T_c = []
        tile_blocks = []
        for ch in range(CH):
            tc = 0
            for b in sg["blocks"]:
                cell = ch * NBLK + b
                nt = int(cell_tiles[cell])
                cell_tile_off[cell] = t_off
                tile_blocks.extend([b] * nt)
                t_off += nt
                tc += nt
            if ch == CH - 1 and sg["pad_tile"]:
                tile_blocks.append(sg["blocks"][0])
                t_off += 1
                tc += 1
            T_c.append(tc)
        first, last = {}, {}
        for i, b in enumerate(tile_blocks):
            if b not in first:
                first[b] = i
            last[b] = i
        sg["T_c"] = T_c
        sg["tile_blocks"] = tile_blocks
        sg["first"] = first
        sg["last"] = last
    S_tiles = t_off
    struct = {"sgs": sgs, "S_tiles": S_tiles, "cell_tiles": cell_tiles,
              "cell_tile_off": cell_tile_off, "chrows": int(chrows),
              "TMAX": max(sg["T"] for sg in sgs)}
    return struct, per_core


META_W = 21             # int16 units per tile: oid 1 + ew 4 + kvx 8 + qx 8


def _per_core_arrays(struct, core, scale):
    """Packed per-core metadata [128, S_tiles*META_W] int16 plus host perm."""
    S_tiles = struct["S_tiles"]
    S = S_tiles * 128
    cell_tile_off = struct["cell_tile_off"]

    oid = np.full(S, PAD_OID, np.float32)
    kvi = np.zeros(S, np.int16)
    ew4 = np.zeros((S, H), np.float32)
    qxi = np.zeros(S, np.int16)

    cnt = core["cnt"]
    cell_edge_off = np.zeros(CH * NBLK + 1, np.int64)
    np.cumsum(cnt, out=cell_edge_off[1:])
    for cell in range(CH * NBLK):
        n = int(cnt[cell])
        if n == 0:
            continue
        e0 = cell_edge_off[cell]
        s0 = cell_tile_off[cell] * 128
        sl = slice(s0, s0 + n)
        el = slice(e0, e0 + n)
        oid[sl] = core["oid"][el].astype(np.float32)
        kvi[sl] = core["dloc"][el].astype(np.int16)
        ew4[sl] = core["ew"][el] * scale
        qxi[sl] = (core["oid"][el] * NBLK + (cell % NBLK)).astype(np.int16)

    def wrap(run_vals):
        w = run_vals.reshape(-1, 16).T
        return np.tile(w, (8, 1))

    # per-sg contiguous regions: [oid(T) | ew(T*4) | kvx(T*8) | prx(T*4)]
    oid_pt = np.ascontiguousarray(
        oid.reshape(S_tiles, 128).T).astype(BF16_NP).view(np.int16)
    ew_pt = ew4.astype(BF16_NP).view(np.int16) \
        .reshape(S_tiles, 128, H).transpose(1, 0, 2)
    meta = np.zeros((128, S_tiles * META_W), np.int16)
    for sg in struct["sgs"]:
        t0, T = sg["tile_off"], sg["T"]
        m0 = t0 * META_W
        meta[:, m0:m0 + T] = oid_pt[:, t0:t0 + T]
        meta[:, m0 + T:m0 + 5 * T] = \
            ew_pt[:, t0:t0 + T, :].reshape(128, T * H)
        meta[:, m0 + 5 * T:m0 + 13 * T] = \
            wrap(kvi[t0 * 128:(t0 + T) * 128])
        meta[:, m0 + 13 * T:m0 + 21 * T] = \
            wrap(qxi[t0 * 128:(t0 + T) * 128])
    return meta


def _build_graph(struct):
    nc = bacc.Bacc()
    S_tiles = struct["S_tiles"]
    TMAX = struct["TMAX"]
    sgs = struct["sgs"]
    CHR = struct["chrows"]

    # x for the kv table, dest-compacted, with a ones row for the bias
    xT = nc.declare_dram_parameter("xT", [D + 1, CH * CHR], BF16,
                                   isOutput=False)
    xTo = nc.declare_dram_parameter("xTo", [D + 1, NB], BF16, isOutput=False)
    wkv = nc.declare_dram_parameter("wkv", [D + 1, 2 * D], BF16, isOutput=False)
    wq = nc.declare_dram_parameter("wq", [D + 1, 2 * D], BF16, isOutput=False)
    wot = nc.declare_dram_parameter("wot", [D, D], F32, isOutput=False)
    boc = nc.declare_dram_parameter("boc", [128, D], F32, isOutput=False)
    gam = nc.declare_dram_parameter("gam", [128, D], F32, isOutput=False)
    THALF = (TMAX + 1) // 2
    iot = nc.declare_dram_parameter("iot", [128, 128, THALF], BF16, isOutput=False)
    idn = nc.declare_dram_parameter("idn", [128, 128], F32, isOutput=False)
    xpb = nc.declare_dram_parameter("xpb", [NB, D], F32, isOutput=False)
    meta = nc.declare_dram_parameter("meta", [128, S_tiles * META_W], I16,
                                     isOutput=False)
    out = nc.declare_dram_parameter("out", [NB, D], F32, isOutput=True)

    kv_tab = [nc.dram_tensor(f"kv_tab{ch}", [CHR, 2 * D], BF16)
              for ch in range(CH)]
    q_tab = nc.dram_tensor("q_tab", [NB, 2 * D], BF16)

    with TileContext(nc) as tc:
        with tc.tile_pool(name="const", bufs=1) as cp:
            wkv_t = cp.tile([D + 1, 2 * D], BF16)
            nc.sync.dma_start(out=wkv_t[:], in_=wkv[:])
            wq_t = cp.tile([D + 1, 2 * D], BF16)
            nc.sync.dma_start(out=wq_t[:], in_=wq[:])
            wot_f = cp.tile([D, D], F32)
            nc.sync.dma_start(out=wot_f[:], in_=wot[:])
            wot_t = cp.tile([D, D], BF16)
            nc.vector.tensor_copy(wot_t[:], wot_f[:])
            boc_t = cp.tile([128, D], F32)
            nc.sync.dma_start(out=boc_t[:], in_=boc[:])
            gam_t = cp.tile([128, D], F32)
            nc.sync.dma_start(out=gam_t[:], in_=gam[:])
            iot_t = cp.tile([128, 128, THALF], BF16)
            nc.sync.dma_start(out=iot_t[:], in_=iot[:])
            idn_t = cp.tile([128, 128], F32)
            nc.sync.dma_start(out=idn_t[:], in_=idn[:])
            poall = cp.tile([128, NBLK, D], F32)       # post-Wo, pre-LN

            # meta prefetch: first 6 sgs' metadata loads issue before the
            # phase-A DMAs queue up on SP, so edge gathers start early
            sg_order = sorted(sgs, key=lambda g: -g["T"])
            mp_cm = tc.tile_pool(name="met", bufs=6)
            mp = mp_cm.__enter__()
            mts = {}

            def load_meta(i):
                g = sg_order[i]
                m = mp.tile([128, TMAX * META_W], I16, tag="mt",
                            name=f"mt{i}")
                nc.sync.dma_start(
                    out=m[:, 0:g["T"] * META_W],
                    in_=meta[:, g["tile_off"] * META_W:
                             (g["tile_off"] + g["T"]) * META_W])
                mts[i] = m

            for i in range(min(6, len(sg_order))):
                load_meta(i)

            # ---- phase A: q (SBUF) first, then kv chunk tables (DRAM).
            # x comes half-packed [130, rows/2]: even rows in partitions
            # 0..64, odd rows in 65..129 (halves the DMA column count).
            BIG = 8192
            with (
                tc.tile_pool(name="pa", bufs=3) as pa,
                tc.tile_pool(name="paq", bufs=1) as paq,
                tc.tile_pool(name="pap", bufs=2, space="PSUM") as pap,
            ):
                # q: one load; block b = columns [b*128, (b+1)*128)
                xq = paq.tile([D + 1, NB], BF16)
                nc.sync.dma_start(out=xq[:], in_=xTo[:])
                QB = 16                      # blocks per psum slab
                for s in range((NBLK + QB - 1) // QB):
                    b0 = s * QB
                    nblk = min(QB, NBLK - b0)
                    ps = pap.tile([128, (SLAB // 128) * 2 * D], F32, tag="ps",
                                  name=f"psq{s}")
                    psv = ps[:].rearrange("p (a d) -> p a d", d=128)
                    for j in range(nblk):
                        b = b0 + j
                        nc.tensor.matmul(
                            out=psv[:, j, :],
                            lhsT=xq[:, b * 128:(b + 1) * 128],
                            rhs=wq_t[:],
                            start=True, stop=True)
                    qb = pa.tile([128, SLAB // 128, 2 * D], BF16, tag="qb")
                    nc.vector.tensor_copy(
                        qb[:, 0:nblk, :].rearrange("p a d -> p (a d)"),
                        ps[:, 0:nblk * 2 * D])
                    # q_tab row p*NBLK + b (p-major: contiguous per partition)
                    nc.gpsimd.dma_start(
                        out=q_tab[:].rearrange("(p a) d -> p a d", p=128)
                            [:, b0:b0 + nblk, :],
                        in_=qb[:, 0:nblk, :])

                # kv: 8192-row DMA slabs, 2048-row GEMM sub-slabs. Partition
                # p holds table rows [n0+rpp8*p, +rpp8); tile j of sub-slab
                # s2 covers rows {rpp8*p + spp*s2 + j} (p-strided lhsT).
                for ch in range(CH):
                    n0 = 0
                    while n0 < CHR:
                        nrows = min(BIG, CHR - n0)
                        rpp8 = nrows // 128
                        xs = pa.tile([D + 1, BIG], BF16, tag="xs")
                        nc.sync.dma_start(
                            out=xs[:, 0:nrows],
                            in_=xT[:, ch * CHR + n0:ch * CHR + n0 + nrows])
                        sb = pa.tile([128, BIG // 128, 2 * D], BF16, tag="sb")
                        nsub = nrows // SLAB
                        spp = rpp8 // nsub       # rows per partition per sub
                        for s2 in range(nsub):
                            ps = pap.tile([128, (SLAB // 128) * 2 * D], F32,
                                          tag="ps", name=f"pkv{ch}_{n0}_{s2}")
                            for j in range(SLAB // 128):
                                r = spp * s2 + j
                                nc.tensor.matmul(
                                    out=ps[:, j * 2 * D:(j + 1) * 2 * D],
                                    lhsT=xs[:, r:nrows:rpp8],
                                    rhs=wkv_t[:],
                                    start=True, stop=True)
                            dst = sb[:, spp * s2:spp * (s2 + 1), :] \
                                .rearrange("p a d -> p (a d)")
                            if s2 % 2 == 0:
                                nc.vector.tensor_copy(
                                    dst, ps[:, 0:SLAB // 128 * 2 * D])
                            else:
                                nc.scalar.copy(
                                    dst, ps[:, 0:SLAB // 128 * 2 * D])
                        nc.gpsimd.dma_start(
                            out=kv_tab[ch][n0:n0 + nrows, :]
                                .rearrange("(p a) d -> p a d", p=128),
                            in_=sb[:, 0:rpp8, :])
                        n0 += nrows

            # ---- phase B: edge processing per super-group
            with (
                tc.tile_pool(name="gat", bufs=2) as gp,
                tc.tile_pool(name="wrk", bufs=2) as wp,
                tc.tile_pool(name="dvi", bufs=1) as dv,
                tc.tile_pool(name="eps", bufs=2) as ep,
                tc.tile_pool(name="bps", bufs=6, space="PSUM") as bp,
                tc.tile_pool(name="tpo", bufs=2, space="PSUM") as tp,
            ):
                for sgi, sg in enumerate(sg_order):
                    T = sg["T"]
                    t0 = sg["tile_off"]
                    mt = mts.pop(sgi)
                    if sgi + 6 < len(sg_order):
                        load_meta(sgi + 6)
                    oid_t = mt[:, 0:T].bitcast(BF16)
                    ew_t = mt[:, T:5 * T].bitcast(BF16) \
                        .rearrange("p (t w) -> p t w", w=H)
                    kvi_t = mt[:, 5 * T:13 * T]
                    qxi_t = mt[:, 13 * T:21 * T]

                    kvg = gp.tile([128, TMAX, 2 * D], BF16, tag="kvg")
                    off = 0
                    for ch in range(CH):
                        tcn = sg["T_c"][ch]
                        if tcn == 0:
                            continue
                        nc.gpsimd.dma_gather(
                            out_ap=kvg[:, off:off + tcn, :],
                            in_ap=kv_tab[ch][:],
                            idxs_ap=kvi_t[:, off * 8:(off + tcn) * 8],
                            num_idxs=tcn * 128,
                            num_idxs_reg=tcn * 128,
                            elem_size=2 * D,
                            single_packet=False)
                        off += tcn
                    qg = gp.tile([128, TMAX, 2 * D], BF16, tag="qg")
                    nc.gpsimd.dma_gather(
                        out_ap=qg[:, 0:T, :],
                        in_ap=q_tab[:],
                        idxs_ap=qxi_t[:],
                        num_idxs=T * 128,
                        num_idxs_reg=T * 128,
                        elem_size=2 * D,
                        single_packet=False)

                    # scores: qk mult (2x bf16) then bf16 tree reduce over d
                    qk = dv.tile([128, TMAX, HD, H], BF16, tag="qk")
                    nc.vector.tensor_tensor(
                        out=qk[:, 0:T, :, :],
                        in0=qg[:, 0:T, 0:D].rearrange("p t (e h) -> p t e h", h=H),
                        in1=kvg[:, 0:T, 0:D].rearrange("p t (e h) -> p t e h", h=H),
                        op=mybir.AluOpType.mult)
                    r8 = dv.tile([128, TMAX, 8, H], BF16, tag="r8")
                    nc.vector.tensor_tensor(
                        out=r8[:, 0:T, :, :], in0=qk[:, 0:T, 0:8, :],
                        in1=qk[:, 0:T, 8:16, :], op=mybir.AluOpType.add)
                    r4 = dv.tile([128, TMAX, 4, H], BF16, tag="r4")
                    nc.vector.tensor_tensor(
                        out=r4[:, 0:T, :, :], in0=r8[:, 0:T, 0:4, :],
                        in1=r8[:, 0:T, 4:8, :], op=mybir.AluOpType.add)
                    r2 = dv.tile([128, TMAX, 2, H], BF16, tag="r2")
                    nc.vector.tensor_tensor(
                        out=r2[:, 0:T, :, :], in0=r4[:, 0:T, 0:2, :],
                        in1=r4[:, 0:T, 2:4, :], op=mybir.AluOpType.add)
                    ws = dv.tile([128, TMAX, 1, H], BF16, tag="ws")
                    nc.vector.tensor_tensor(
                        out=ws[:, 0:T, :, :], in0=r2[:, 0:T, 0:1, :],
                        in1=r2[:, 0:T, 1:2, :], op=mybir.AluOpType.add)
                    wsb = dv.tile([128, TMAX, H], BF16, tag="wsb")
                    nc.vector.tensor_tensor(
                        out=wsb[:, 0:T, :], in0=ws[:, 0:T, 0, :],
                        in1=qg[:, 0:T, D:D + H], op=mybir.AluOpType.add)
                    wse = wp.tile([128, TMAX, H], BF16, tag="wse")
                    nc.vector.tensor_tensor(
                        out=wse[:, 0:T, :],
                        in0=wsb[:, 0:T, :],
                        in1=ew_t[:],
                        op=mybir.AluOpType.mult)
                    ctb = wp.tile([128, TMAX, D + H], BF16, tag="ctb")
                    nc.scalar.activation(
                        out=ctb[:, 0:T, D:D + H], in_=wse[:, 0:T, :],
                        func=mybir.ActivationFunctionType.Exp)
                    nc.vector.tensor_tensor(
                        out=ctb[:, 0:T, 0:D].rearrange("p t (e h) -> p t e h", h=H),
                        in0=kvg[:, 0:T, D:2 * D].rearrange("p t (e h) -> p t e h", h=H),
                        in1=ctb[:, 0:T, D:D + H].rearrange("p t (o h) -> p t o h", o=1)
                            .to_broadcast([128, T, HD, H]),
                        op=mybir.AluOpType.mult)

                    # sel one-hots + scatter matmuls into per-block psums
                    # (two instrs so the iota const is only TMAX/2 deep)
                    sel = wp.tile([128, 128, TMAX], BF16, tag="sel")
                    ha = T // 2
                    for s0, sn in ((0, ha), (ha, T - ha)):
                        nc.vector.tensor_tensor(
                            out=sel[:, :, s0:s0 + sn],
                            in0=oid_t[:, s0:s0 + sn]
                                .rearrange("p (o t) -> p o t", o=1)
                                .to_broadcast([128, 128, sn]),
                            in1=iot_t[:, :, 0:sn],
                            op=mybir.AluOpType.is_equal)
                    psums = {}
                    for i, b in enumerate(sg["blocks"]):
                        psums[b] = bp.tile([128, D + H], F32, tag="bps",
                                           name=f"bps{b}")[:]
                    for i, b in enumerate(sg["tile_blocks"]):
                        nc.tensor.matmul(
                            out=psums[b][:],
                            lhsT=sel[:, :, i],
                            rhs=ctb[:, i, :],
                            start=(sg["first"][b] == i),
                            stop=(sg["last"][b] == i))

                    # per-sg epilogue: divide by z, transpose, Wo -> poall
                    blocks = sg["blocks"]
                    NBk = len(blocks)
                    zr = ep.tile([128, BCAP, H], F32, tag="zr")
                    for i, b in enumerate(blocks):
                        nc.vector.tensor_scalar_add(
                            zr[:, i, :], psums[b][:, D:D + H], 1e-16)
                    nc.vector.reciprocal(zr[:, 0:NBk, :], zr[:, 0:NBk, :])
                    vals = ep.tile([128, BCAP, D], F32, tag="vals")
                    for i, b in enumerate(blocks):
                        nc.vector.tensor_tensor(
                            out=vals[:, i, :].rearrange("p (e h) -> p e h", h=H),
                            in0=psums[b][:, 0:D].rearrange("p (e h) -> p e h", h=H),
                            in1=zr[:, i, :].rearrange("p (o h) -> p o h", o=1)
                                .to_broadcast([128, HD, H]),
                            op=mybir.AluOpType.mult)
                    for i, b in enumerate(blocks):
                        tpo = tp.tile([128, 192], F32, tag="tpo", name=f"tpo{i}")
                        pt = tpo[0:D, 0:128]
                        po = tpo[:, 128:192]
                        nc.tensor.transpose(out=pt, in_=vals[:, i, :],
                                            identity=idn_t[:])
                        vT = ep.tile([D, 128], BF16, tag="vT", name=f"vT{i}")
                        nc.scalar.copy(vT[:], pt)
                        nc.tensor.matmul(out=po, lhsT=vT[:], rhs=wot_t[:],
                                         start=True, stop=True)
                        nc.scalar.copy(poall[:, b, :], po)

            mp_cm.__exit__(None, None, None)

            # ---- phase C: batched LayerNorm + residual, two half-batches.
            # xpb/out use the p-major layout: DRAM row p*NBLK + a holds the
            # data for permuted node a*128 + p (1 DMA descriptor/partition).
            with tc.tile_pool(name="fin", bufs=2) as fp:
                for b0, b1 in ((0, NBLK // 2), (NBLK // 2, NBLK)):
                    nb = b1 - b0
                    pslab = poall[:, b0:b1, :]
                    nmu = fp.tile([128, NBLK // 2 + 1], F32, tag="nmu")
                    nc.vector.tensor_reduce(
                        out=nmu[:, 0:nb], in_=pslab,
                        axis=mybir.AxisListType.X, op=mybir.AluOpType.add)
                    nc.vector.tensor_scalar_mul(
                        nmu[:, 0:nb], nmu[:, 0:nb], -1.0 / D)
                    ct = fp.tile([128, NBLK // 2 + 1, D], F32, tag="ct")
                    nc.vector.tensor_tensor(
                        out=ct[:, 0:nb, :], in0=pslab,
                        in1=nmu[:, 0:nb].rearrange("p (b o) -> p b o", o=1)
                            .to_broadcast([128, nb, D]),
                        op=mybir.AluOpType.add)
                    nc.gpsimd.tensor_tensor(
                        out=ct[:, 0:nb, :], in0=ct[:, 0:nb, :],
                        in1=boc_t[:].rearrange("p (o d) -> p o d", o=1)
                            .to_broadcast([128, nb, D]),
                        op=mybir.AluOpType.add)
                    sq = fp.tile([128, NBLK // 2 + 1, D], F32, tag="sq")
                    nc.gpsimd.tensor_tensor(
                        out=sq[:, 0:nb, :], in0=ct[:, 0:nb, :],
                        in1=ct[:, 0:nb, :], op=mybir.AluOpType.mult)
                    v1 = fp.tile([128, NBLK // 2 + 1], F32, tag="v1")
                    nc.vector.tensor_reduce(
                        out=v1[:, 0:nb], in_=sq[:, 0:nb, :],
                        axis=mybir.AxisListType.X, op=mybir.AluOpType.add)
                    nc.vector.tensor_scalar(
                        out=v1[:, 0:nb], in0=v1[:, 0:nb],
                        scalar1=1.0 / D, scalar2=LN_EPS,
                        op0=mybir.AluOpType.mult, op1=mybir.AluOpType.add)
                    # rstd = exp(-0.5*ln(var+eps)): stays in the exp/ln table
                    lnv = fp.tile([128, NBLK // 2 + 1], F32, tag="lnv")
                    nc.scalar.activation(
                        out=lnv[:, 0:nb], in_=v1[:, 0:nb],
                        func=mybir.ActivationFunctionType.Ln)
                    rstd = fp.tile([128, NBLK // 2 + 1], F32, tag="rstd")
                    nc.scalar.activation(
                        out=rstd[:, 0:nb], in_=lnv[:, 0:nb], scale=-0.5,
                        func=mybir.ActivationFunctionType.Exp)
                    xb = fp.tile([128, NBLK // 2 + 1, D], F32, tag="xb")
                    nc.sync.dma_start(
                        out=xb[:, 0:nb, :],
                        in_=xpb[:].rearrange("(p a) d -> p a d", p=128)
                            [:, b0:b1, :])
                    ot = fp.tile([128, NBLK // 2 + 1, D], F32, tag="ot")
                    nc.vector.tensor_tensor(
                        out=ot[:, 0:nb, :], in0=ct[:, 0:nb, :],
                        in1=rstd[:, 0:nb].rearrange("p (b o) -> p b o", o=1)
                            .to_broadcast([128, nb, D]),
                        op=mybir.AluOpType.mult)
                    nc.gpsimd.tensor_tensor(
                        out=ot[:, 0:nb, :], in0=ot[:, 0:nb, :],
                        in1=gam_t[:].rearrange("p (o d) -> p o d", o=1)
                            .to_broadcast([128, nb, D]),
                        op=mybir.AluOpType.mult)
                    nc.gpsimd.tensor_tensor(
                        out=ot[:, 0:nb, :], in0=ot[:, 0:nb, :],
                        in1=xb[:, 0:nb, :], op=mybir.AluOpType.add)
                    nc.sync.dma_start(
                        out=out[:].rearrange("(p a) d -> p a d", p=128)
                            [:, b0:b1, :],
                        in_=ot[:, 0:nb, :])
    return nc


def kernel(x, edge_index, edge_weights, Wq, bq, Wk, bk, Wv, bv, Wo, bo,
           gamma, beta):
    x = np.asarray(x, np.float32)
    edge_index = np.asarray(edge_index)
    edge_weights = np.asarray(edge_weights, np.float32)
    origins = np.asarray(edge_index[0], np.int64)
    dests = np.asarray(edge_index[1], np.int64)

    struct, per_core = _build_structure(origins, dests, edge_weights)
    nc = _build_graph(struct)
    nc.finalize()

    # shared (replicated) host arrays.
    # v uses (e, h)-interleaved layout; k and q use it too so the d-axis
    # tree reduce groups by head with h innermost.
    vperm = (np.arange(H)[None, :] * HD + np.arange(HD)[:, None]).ravel()
    Wkf = np.asarray(Wk, np.float32)
    Wvf = np.asarray(Wv, np.float32)
    Wqf = np.asarray(Wq, np.float32)
    bkf = np.asarray(bk, np.float32)
    bvf = np.asarray(bv, np.float32)
    bqf = np.asarray(bq, np.float32)
    wkv = np.zeros((D + 1, 2 * D), np.float32)
    wkv[:D, :D] = Wkf.T[:, vperm]
    wkv[:D, D:] = Wvf.T[:, vperm]
    wkv = wkv.astype(BF16_NP)          # k/v biases fold into q.bk / boc
    # q gets H extra columns projecting x onto sum_d Wq[d,:]*bk[d] per head
    # (score = q.k_nobias + q.bk, and q.bk = x @ wqx_h + bq.bk_h)
    wq_h = np.zeros((D + 1, 2 * D), np.float32)
    wq_h[:D, 0:D] = Wqf.T[:, vperm]
    wq_h[D, 0:D] = bqf[vperm]
    for h in range(H):
        dims = np.arange(HD) + h * HD          # original k dims of head h
        wq_h[:D, D + h] = Wqf.T[:, dims] @ bkf[dims]
        wq_h[D, D + h] = bqf[dims] @ bkf[dims]
    wq_h = wq_h.astype(BF16_NP)
    wot = np.ascontiguousarray(np.asarray(Wo, np.float32).T[vperm, :])
    # bv contributes bv @ Wo.T to every output row (sum of attn = 1)
    bo2 = np.asarray(bo, np.float32) + bvf @ np.asarray(Wo, np.float32).T
    boc = np.tile((bo2 - bo2.mean())[None, :], (128, 1)).astype(np.float32)
    gam_t = np.tile(np.asarray(gamma, np.float32)[None, :], (128, 1))
    THALF = (struct["TMAX"] + 1) // 2
    iot = np.tile(np.arange(128, dtype=np.float32)[None, :, None],
                  (128, 1, THALF)).astype(BF16_NP)
    idn = np.eye(128, dtype=np.float32)

    scale = HD ** -0.5
    in_maps = []
    for c in range(NCORES):
        core = per_core[c]
        meta = _per_core_arrays(struct, core, scale)
        perm = core["perm"]
        xc = x[c * NOWN:(c + 1) * NOWN]
        xp = np.zeros((NB, D), np.float32)
        valid = perm >= 0
        xp[valid] = xc[perm[valid]]
        xTo = np.zeros((D + 1, NB), np.float32)
        xTo[:D] = xp.T
        xTo[D] = 1.0
        xTo = xTo.astype(BF16_NP)
        # kv x, dest-compacted and half-packed by table-row parity; chunk
        # ch's table rows are ranks [bounds[ch], bounds[ch+1])
        CHR = struct["chrows"]
        ucol = core["ucol"]
        bounds = core["bounds"]
        tabx = np.zeros((CH * CHR, D), np.float32)
        for ch in range(CH):
            n = int(bounds[ch + 1] - bounds[ch])
            tabx[ch * CHR:ch * CHR + n] = x[ucol[bounds[ch]:bounds[ch + 1]]]
        xTc = np.zeros((D + 1, CH * CHR), np.float32)
        xTc[:D] = tabx.T
        xTc[D] = 1.0
        xTc = xTc.astype(BF16_NP)
        # p-major: row p*NBLK + a  <-  permuted node a*128 + p
        xpb = np.ascontiguousarray(
            (xp + np.asarray(beta, np.float32)[None, :])
            .reshape(NBLK, 128, D).transpose(1, 0, 2)).reshape(NB, D)
        in_maps.append({
            "xT": xTc, "xTo": xTo, "wkv": wkv,
            "wq": wq_h, "wot": wot,
            "boc": boc, "gam": gam_t, "iot": iot, "idn": idn,
            "xpb": xpb, "meta": meta,
        })

    global LAST_SIM_NS
    if SIMULATE_COST:
        from concourse import bass_interp
        sim = bass_interp.CoreSim(nc, no_exec=True, publish_trace=False)
        sim.event_loop()
        LAST_SIM_NS = int(sim.time)

    res = run_bass_kernel_spmd(nc, in_maps, core_ids=list(range(NCORES)),
                               trace=TRACE)
    global LAST_RESULT
    LAST_RESULT = res
    full = np.zeros((N, D), np.float32)
    for c in range(NCORES):
        o = np.asarray(res.results[c]["out"])
        # p-major: row p*NBLK + a holds permuted node a*128 + p
        o = o.reshape(128, NBLK, D).transpose(1, 0, 2).reshape(NB, D)
        perm = per_core[c]["perm"]
        valid = perm >= 0
        full[c * NOWN + perm[valid]] = o[valid]
    return full


TRACE = False
SIMULATE_COST = False
LAST_RESULT = None
LAST_SIM_NS = None


# revision 69
# speedup vs baseline: 1.4510x; 1.0104x over previous
"""Trainium2 Bass kernel for BaseDependentAttentionLayer (GNN message passing).

Strategy (8 NeuronCores, SPMD, no collectives):
  - Nodes sharded by origin: core c owns nodes [c*12500, (c+1)*12500).
  - Origins are permuted into 98 blocks of <=128 per core, bin-packed so
    each (dest-chunk, block) cell holds ~<=512 edges -> near-zero tile pad.
  - Each core computes the bf16 [k|v] row table for ALL nodes (4 chunk
    tensors in DRAM so edge gathers overlap the build) and q for its own
    nodes (kept in SBUF).
  - Edges bucketed by (dest-chunk, origin-block), padded to 128-edge tiles;
    per-edge k|v fetched with dma_gather (int16 idx, 256B rows).
  - Per-slot q is NOT gathered: a pair-one-hot table (129x129 rows of
    [onehot(a)|onehot(b)], 512B) is gathered transpose-style to give
    selT[origin, slot]; one matmul per tile against the q block broadcasts
    q to slots (PSUM), copied to SBUF bf16 by ACT.
  - Segment softmax runs without max-subtraction (shift-invariant; values
    bounded); segment sums via 0/1 selection-matrix matmuls into per-block
    PSUM accumulators; sel is built by DVE is_equal at 2 elem/cycle.
  - Scores reduce via a bf16 tree-fold (2x DVE rate). exp writes directly
    into the ctb tail. LayerNorm+residual is deferred to one batched final
    phase; rstd = exp(-0.5*ln(var+eps)) so ACT never swaps tables.
"""

import sys

sys.path.insert(0, "/opt/trn_rl_repo")

import numpy as np
import ml_dtypes

import concourse.bass as bass
import concourse.bacc as bacc
import concourse.mybir as mybir
from concourse.tile import TileContext
from concourse.bass_utils import run_bass_kernel_spmd

N = 100000
E = 1600000
D = 64
H = 4
HD = 16
NCORES = 8
NOWN = 12500            # nodes owned per core
NBLK = 98               # 128-node origin blocks per core
NB = NBLK * 128         # 12544 padded own nodes
NT = 100352             # padded global table rows (= 4 * 25088)
CH = 4                  # dest chunks (int16 gather index limit)
CHROWS = NT // CH       # 25088
TCAP = 64               # max tiles per super-group (psum / sbuf budget)
BCAP = 4                # max blocks per super-group
LN_EPS = 1e-5
PAD_OID = 200.0         # origin-id sentinel for pad slots (matches no node)
POT_SYM = 129           # pair-one-hot symbols (128 origins + zero pad)

F32 = mybir.dt.float32
BF16 = mybir.dt.bfloat16
I16 = mybir.dt.int16
BF16_NP = ml_dtypes.bfloat16

SLAB = 2048             # phase-A rows per slab


def _balance_blocks(chunk_of_edge_dest, local_origin):
    """Assign local origins to NBLK blocks (<=128 each), balancing the
    per-(chunk, block) edge counts with 4-D LPT so cells pack near 512.

    Returns perm: perm[new_node_index] = local_origin (block b owns
    perm[b*128:(b+1)*128] entries; tail entries may be -1 = unused)."""
    cnt = np.zeros((NOWN, CH), np.int64)
    np.add.at(cnt, (local_origin, chunk_of_edge_dest), 1)
    deg = cnt.sum(1)
    order = np.argsort(-deg, kind="stable")
    CAP = 4 * 128            # capped blocks: every cell fits in 4 tiles
    NOVF = 2                 # overflow blocks (uncapped) at the end
    NCAPB = NBLK - NOVF
    bsum = np.zeros((NBLK, CH), np.int64)
    bcnt = np.zeros(NBLK, np.int64)
    assign = np.zeros(NOWN, np.int64)
    # heaviest origins soak into the overflow blocks so the capped blocks'
    # total fits under NCAPB*CAP per chunk
    for i, o in enumerate(order[:NOVF * 128]):
        b = NCAPB + i % NOVF
        assign[o] = b
        bsum[b] += cnt[o]
        bcnt[b] += 1
    for o in order[NOVF * 128:]:
        c = cnt[o]
        cand = (bsum[:NCAPB] + c[None, :]).max(1).astype(np.float64)
        open_ = bcnt[:NCAPB] < 128
        feas = (cand <= CAP) & open_
        if feas.any():
            # worst-fit: keep all blocks growing evenly (tight 4-D packing)
            score = np.where(feas, cand, np.inf)
        else:
            score = np.where(open_, cand, np.inf)
        b = int(np.argmin(score))
        assign[o] = b
        bsum[b] += c
        bcnt[b] += 1
    perm = np.full(NBLK * 128, -1, np.int64)
    pos = 0
    for b in range(NBLK):
        members = np.nonzero(assign == b)[0]
        perm[b * 128:b * 128 + len(members)] = members
    return perm, assign


def _build_structure(origins, dests, edge_weights):
    """Global (core-independent) tile structure + per-core slot data.

    Dest nodes are compacted per core (only referenced nodes get kv-table
    rows); chunking is by compacted rank."""
    owner = origins // NOWN
    # pass 1: per-core unique dests -> table size (shared across cores)
    ucols = []
    for c in range(NCORES):
        ucols.append(np.unique(dests[owner == c]))
    max_u = max(len(u) for u in ucols)
    chrows = -(-max_u // (4 * SLAB)) * SLAB      # per-chunk rows, 2048-mult
    assert chrows <= 32000
    per_core = []
    cell_cnt = np.zeros((NCORES, CH * NBLK), np.int64)
    for c in range(NCORES):
        m = owner == c
        o = (origins[m] - c * NOWN).astype(np.int64)
        d = dests[m].astype(np.int64)
        ew = edge_weights[m]
        rank = np.searchsorted(ucols[c], d)
        # chunk boundaries equalize EDGES per chunk (cells pack to ~512);
        # each chunk's rank span must still fit the CHR-row table
        rdeg = np.bincount(rank, minlength=len(ucols[c]))
        cum = np.cumsum(rdeg)
        nb_ = [0] + [int(np.searchsorted(cum, cum[-1] * k // CH))
                     for k in (1, 2, 3)] + [len(ucols[c])]
        bounds = np.asarray(nb_, np.int64)
        assert (np.diff(bounds) <= chrows).all()
        chunk = np.searchsorted(bounds[1:-1], rank, side="right")
        perm, assign = _balance_blocks(chunk, o)
        # origin -> (block, slot-in-block) position
        opos = np.zeros(NOWN, np.int64)
        valid = perm >= 0
        opos[perm[valid]] = np.nonzero(valid)[0]
        p = opos[o]                       # position in permuted node space
        blk = p >> 7
        cell = chunk * NBLK + blk
        order = np.argsort(cell, kind="stable")
        cnt = np.bincount(cell, minlength=CH * NBLK)
        cell_cnt[c] = cnt
        per_core.append({
            "perm": perm, "cell": cell[order], "oid": (p & 127)[order],
            "dloc": (rank - bounds[chunk])[order], "ew": ew[order],
            "cnt": cnt, "ucol": ucols[c], "bounds": bounds,
        })
    cmax = cell_cnt.max(0)
    cell_tiles = np.maximum((cmax + 127) // 128, 1)   # [CH*NBLK]

    # super-groups: consecutive blocks, <=BCAP blocks, <=TCAP tiles, T even
    blk_tiles = cell_tiles.reshape(CH, NBLK).sum(0)   # tiles per block
    sgs = []
    start = 0
    while start < NBLK:
        nb, t = 0, 0
        while (start + nb < NBLK and nb < BCAP
               and t + blk_tiles[start + nb] <= TCAP):
            t += blk_tiles[start + nb]
            nb += 1
        if nb == 0:
            nb, t = 1, int(blk_tiles[start])
        blocks = list(range(start, start + nb))
        pad_tile = t % 2                # keep T even for the selT gather
        sgs.append({"blocks": blocks, "T": t + pad_tile, "pad_tile": pad_tile})
        start += nb

    # stream order: sg -> chunk -> block; the optional pad tile sits at the
    # end of the last chunk segment and belongs to the sg's first block.
    t_off = 0
    cell_tile_off = np.zeros(CH * NBLK, np.int64)
    for sg in sgs:
        sg["tile_off"] = t_off
        T_c = []
        tile_blocks = []
        for ch in range(CH):
            tc = 0
            for b in sg["blocks"]:
                cell = ch * NBLK + b
                nt = int(cell_tiles[cell])
                cell_tile_off[cell] = t_off
                tile_blocks.extend([b] * nt)
                t_off += nt
                tc += nt
            if ch == CH - 1 and sg["pad_tile"]:
                tile_blocks.append(sg["blocks"][0])
                t_off += 1
                tc += 1
            T_c.append(tc)
        first, last = {}, {}
        for i, b in enumerate(tile_blocks):
            if b not in first:
                first[b] = i
            last[b] = i
        sg["T_c"] = T_c
        sg["tile_blocks"] = tile_blocks
        sg["first"] = first
        sg["last"] = last
    S_tiles = t_off
    struct = {"sgs": sgs, "S_tiles": S_tiles, "cell_tiles": cell_tiles,
              "cell_tile_off": cell_tile_off, "chrows": int(chrows),
              "TMAX": max(sg["T"] for sg in sgs)}
    return struct, per_core


META_W = 21             # int16 units per tile: oid 1 + ew 4 + kvx 8 + qx 8


def _per_core_arrays(struct, core, scale):
    """Packed per-core metadata [128, S_tiles*META_W] int16 plus host perm."""
    S_tiles = struct["S_tiles"]
    S = S_tiles * 128
    cell_tile_off = struct["cell_tile_off"]

    oid = np.full(S, PAD_OID, np.float32)
    kvi = np.zeros(S, np.int16)
    ew4 = np.zeros((S, H), np.float32)
    qxi = np.zeros(S, np.int16)

    cnt = core["cnt"]
    cell_edge_off = np.zeros(CH * NBLK + 1, np.int64)
    np.cumsum(cnt, out=cell_edge_off[1:])
    for cell in range(CH * NBLK):
        n = int(cnt[cell])
        if n == 0:
            continue
        e0 = cell_edge_off[cell]
        s0 = cell_tile_off[cell] * 128
        sl = slice(s0, s0 + n)
        el = slice(e0, e0 + n)
        oid[sl] = core["oid"][el].astype(np.float32)
        kvi[sl] = core["dloc"][el].astype(np.int16)
        ew4[sl] = core["ew"][el] * scale
        qxi[sl] = (core["oid"][el] * NBLK + (cell % NBLK)).astype(np.int16)

    def wrap(run_vals):
        w = run_vals.reshape(-1, 16).T
        return np.tile(w, (8, 1))

    # per-sg contiguous regions: [oid(T) | ew(T*4) | kvx(T*8) | prx(T*4)]
    oid_pt = np.ascontiguousarray(
        oid.reshape(S_tiles, 128).T).astype(BF16_NP).view(np.int16)
    ew_pt = ew4.astype(BF16_NP).view(np.int16) \
        .reshape(S_tiles, 128, H).transpose(1, 0, 2)
    meta = np.zeros((128, S_tiles * META_W), np.int16)
    for sg in struct["sgs"]:
        t0, T = sg["tile_off"], sg["T"]
        m0 = t0 * META_W
        meta[:, m0:m0 + T] = oid_pt[:, t0:t0 + T]
        meta[:, m0 + T:m0 + 5 * T] = \
            ew_pt[:, t0:t0 + T, :].reshape(128, T * H)
        meta[:, m0 + 5 * T:m0 + 13 * T] = \
            wrap(kvi[t0 * 128:(t0 + T) * 128])
        meta[:, m0 + 13 * T:m0 + 21 * T] = \
            wrap(qxi[t0 * 128:(t0 + T) * 128])
    return meta


def _build_graph(struct):
    nc = bacc.Bacc()
    S_tiles = struct["S_tiles"]
    TMAX = struct["TMAX"]
    sgs = struct["sgs"]
    CHR = struct["chrows"]

    # x for the kv table, dest-compacted, with a ones row for the bias
    xT = nc.declare_dram_parameter("xT", [D + 1, CH * CHR], BF16,
                                   isOutput=False)
    xTo = nc.declare_dram_parameter("xTo", [D + 1, NB], BF16, isOutput=False)
    wkv = nc.declare_dram_parameter("wkv", [D + 1, 2 * D], BF16, isOutput=False)
    wq = nc.declare_dram_parameter("wq", [D + 1, 2 * D], BF16, isOutput=False)
    wot = nc.declare_dram_parameter("wot", [D, D], F32, isOutput=False)
    boc = nc.declare_dram_parameter("boc", [128, D], F32, isOutput=False)
    gam = nc.declare_dram_parameter("gam", [128, D], F32, isOutput=False)
    THALF = (TMAX + 1) // 2
    iot = nc.declare_dram_parameter("iot", [128, 128, THALF], BF16, isOutput=False)
    idn = nc.declare_dram_parameter("idn", [128, 128], F32, isOutput=False)
    xpb = nc.declare_dram_parameter("xpb", [NB, D], F32, isOutput=False)
    meta = nc.declare_dram_parameter("meta", [128, S_tiles * META_W], I16,
                                     isOutput=False)
    out = nc.declare_dram_parameter("out", [NB, D], F32, isOutput=True)

    kv_tab = [nc.dram_tensor(f"kv_tab{ch}", [CHR, 2 * D], BF16)
              for ch in range(CH)]
    q_tab = nc.dram_tensor("q_tab", [NB, 2 * D], BF16)

    with TileContext(nc) as tc:
        with tc.tile_pool(name="const", bufs=1) as cp:
            wkv_t = cp.tile([D + 1, 2 * D], BF16)
            nc.sync.dma_start(out=wkv_t[:], in_=wkv[:])
            wq_t = cp.tile([D + 1, 2 * D], BF16)
            nc.sync.dma_start(out=wq_t[:], in_=wq[:])
            wot_f = cp.tile([D, D], F32)
            nc.sync.dma_start(out=wot_f[:], in_=wot[:])
            wot_t = cp.tile([D, D], BF16)
            nc.vector.tensor_copy(wot_t[:], wot_f[:])
            boc_t = cp.tile([128, D], F32)
            nc.sync.dma_start(out=boc_t[:], in_=boc[:])
            gam_t = cp.tile([128, D], F32)
            nc.sync.dma_start(out=gam_t[:], in_=gam[:])
            iot_t = cp.tile([128, 128, THALF], BF16)
            nc.sync.dma_start(out=iot_t[:], in_=iot[:])
            idn_t = cp.tile([128, 128], F32)
            nc.sync.dma_start(out=idn_t[:], in_=idn[:])
            poall = cp.tile([128, NBLK, D], F32)       # post-Wo, pre-LN

            # meta prefetch: first 6 sgs' metadata loads issue before the
            # phase-A DMAs queue up on SP, so edge gathers start early
            sg_order = sorted(sgs, key=lambda g: -g["T"])
            mp_cm = tc.tile_pool(name="met", bufs=6)
            mp = mp_cm.__enter__()
            mts = {}

            def load_meta(i):
                g = sg_order[i]
                m = mp.tile([128, TMAX * META_W], I16, tag="mt",
                            name=f"mt{i}")
                nc.sync.dma_start(
                    out=m[:, 0:g["T"] * META_W],
                    in_=meta[:, g["tile_off"] * META_W:
                             (g["tile_off"] + g["T"]) * META_W])
                mts[i] = m

            for i in range(min(6, len(sg_order))):
                load_meta(i)

            # ---- phase A: q (SBUF) first, then kv chunk tables (DRAM).
            # x comes half-packed [130, rows/2]: even rows in partitions
            # 0..64, odd rows in 65..129 (halves the DMA column count).
            BIG = 8192
            with (
                tc.tile_pool(name="pa", bufs=3) as pa,
                tc.tile_pool(name="paq", bufs=1) as paq,
                tc.tile_pool(name="pap", bufs=2, space="PSUM") as pap,
            ):
                # q: one load; block b = columns [b*128, (b+1)*128)
                xq = paq.tile([D + 1, NB], BF16)
                nc.sync.dma_start(out=xq[:], in_=xTo[:])
                QB = 16                      # blocks per psum slab
                for s in range((NBLK + QB - 1) // QB):
                    b0 = s * QB
                    nblk = min(QB, NBLK - b0)
                    ps = pap.tile([128, (SLAB // 128) * 2 * D], F32, tag="ps",
                                  name=f"psq{s}")
                    psv = ps[:].rearrange("p (a d) -> p a d", d=128)
                    for j in range(nblk):
                        b = b0 + j
                        nc.tensor.matmul(
                            out=psv[:, j, :],
                            lhsT=xq[:, b * 128:(b + 1) * 128],
                            rhs=wq_t[:],
                            start=True, stop=True)
                    qb = pa.tile([128, SLAB // 128, 2 * D], BF16, tag="qb")
                    nc.vector.tensor_copy(
                        qb[:, 0:nblk, :].rearrange("p a d -> p (a d)"),
                        ps[:, 0:nblk * 2 * D])
                    # q_tab row p*NBLK + b (p-major: contiguous per partition)
                    nc.gpsimd.dma_start(
                        out=q_tab[:].rearrange("(p a) d -> p a d", p=128)
                            [:, b0:b0 + nblk, :],
                        in_=qb[:, 0:nblk, :])

                # kv: 8192-row DMA slabs, 2048-row GEMM sub-slabs. Partition
                # p holds table rows [n0+rpp8*p, +rpp8); tile j of sub-slab
                # s2 covers rows {rpp8*p + spp*s2 + j} (p-strided lhsT).
                for ch in range(CH):
                    n0 = 0
                    while n0 < CHR:
                        nrows = min(BIG, CHR - n0)
                        rpp8 = nrows // 128
                        xs = pa.tile([D + 1, BIG], BF16, tag="xs")
                        nc.sync.dma_start(
                            out=xs[:, 0:nrows],
                            in_=xT[:, ch * CHR + n0:ch * CHR + n0 + nrows])
                        sb = pa.tile([128, BIG // 128, 2 * D], BF16, tag="sb")
                        nsub = nrows // SLAB
                        spp = rpp8 // nsub       # rows per partition per sub
                        for s2 in range(nsub):
                            ps = pap.tile([128, (SLAB // 128) * 2 * D], F32,
                                          tag="ps", name=f"pkv{ch}_{n0}_{s2}")
                            for j in range(SLAB // 128):
                                r = spp * s2 + j
                                nc.tensor.matmul(
                                    out=ps[:, j * 2 * D:(j + 1) * 2 * D],
                                    lhsT=xs[:, r:nrows:rpp8],
                                    rhs=wkv_t[:],
                                    start=True, stop=True)
                            dst = sb[:, spp * s2:spp * (s2 + 1), :] \
                                .rearrange("p a d -> p (a d)")
                            if s2 % 2 == 0:
                                nc.vector.tensor_copy(
                                    dst, ps[:, 0:SLAB // 128 * 2 * D])
                            else:
                                nc.scalar.copy(
                                    dst, ps[:, 0:SLAB // 128 * 2 * D])
                        nc.gpsimd.dma_start(
                            out=kv_tab[ch][n0:n0 + nrows, :]
                                .rearrange("(p a) d -> p a d", p=128),
                            in_=sb[:, 0:rpp8, :])
                        n0 += nrows

            # ---- phase B: edge processing per super-group
            with (
                tc.tile_pool(name="gat", bufs=2) as gp,
                tc.tile_pool(name="wrk", bufs=2) as wp,
                tc.tile_pool(name="dvi", bufs=1) as dv,
                tc.tile_pool(name="eps", bufs=2) as ep,
                tc.tile_pool(name="bps", bufs=6, space="PSUM") as bp,
                tc.tile_pool(name="tpo", bufs=2, space="PSUM") as tp,
            ):
                for sgi, sg in enumerate(sg_order):
                    T = sg["T"]
                    t0 = sg["tile_off"]
                    mt = mts.pop(sgi)
                    if sgi + 6 < len(sg_order):
                        load_meta(sgi + 6)
                    oid_t = mt[:, 0:T].bitcast(BF16)
                    ew_t = mt[:, T:5 * T].bitcast(BF16) \
                        .rearrange("p (t w) -> p t w", w=H)
                    kvi_t = mt[:, 5 * T:13 * T]
                    qxi_t = mt[:, 13 * T:21 * T]

                    kvg = gp.tile([128, TMAX, 2 * D], BF16, tag="kvg")
                    off = 0
                    for ch in range(CH):
                        tcn = sg["T_c"][ch]
                        if tcn == 0:
                            continue
                        nc.gpsimd.dma_gather(
                            out_ap=kvg[:, off:off + tcn, :],
                            in_ap=kv_tab[ch][:],
                            idxs_ap=kvi_t[:, off * 8:(off + tcn) * 8],
                            num_idxs=tcn * 128,
                            num_idxs_reg=tcn * 128,
                            elem_size=2 * D,
                            single_packet=False)
                        off += tcn
                    qg = gp.tile([128, TMAX, 2 * D], BF16, tag="qg")
                    nc.gpsimd.dma_gather(
                        out_ap=qg[:, 0:T, :],
                        in_ap=q_tab[:],
                        idxs_ap=qxi_t[:],
                        num_idxs=T * 128,
                        num_idxs_reg=T * 128,
                        elem_size=2 * D,
                        single_packet=False)

                    # scores: qk mult (2x bf16) then bf16 tree reduce over d
                    qk = dv.tile([128, TMAX, HD, H], BF16, tag="qk")
                    nc.vector.tensor_tensor(
                        out=qk[:, 0:T, :, :],
                        in0=qg[:, 0:T, 0:D].rearrange("p t (e h) -> p t e h", h=H),
                        in1=kvg[:, 0:T, 0:D].rearrange("p t (e h) -> p t e h", h=H),
                        op=mybir.AluOpType.mult)
                    r8 = dv.tile([128, TMAX, 8, H], BF16, tag="r8")
                    nc.vector.tensor_tensor(
                        out=r8[:, 0:T, :, :], in0=qk[:, 0:T, 0:8, :],
                        in1=qk[:, 0:T, 8:16, :], op=mybir.AluOpType.add)
                    r4 = dv.tile([128, TMAX, 4, H], BF16, tag="r4")
                    nc.vector.tensor_tensor(
                        out=r4[:, 0:T, :, :], in0=r8[:, 0:T, 0:4, :],
                        in1=r8[:, 0:T, 4:8, :], op=mybir.AluOpType.add)
                    r2 = dv.tile([128, TMAX, 2, H], BF16, tag="r2")
                    nc.vector.tensor_tensor(
                        out=r2[:, 0:T, :, :], in0=r4[:, 0:T, 0:2, :],
                        in1=r4[:, 0:T, 2:4, :], op=mybir.AluOpType.add)
                    ws = dv.tile([128, TMAX, 1, H], BF16, tag="ws")
                    nc.vector.tensor_tensor(
                        out=ws[:, 0:T, :, :], in0=r2[:, 0:T, 0:1, :],
                        in1=r2[:, 0:T, 1:2, :], op=mybir.AluOpType.add)
                    wsb = dv.tile([128, TMAX, H], BF16, tag="wsb")
                    nc.vector.tensor_tensor(
                        out=wsb[:, 0:T, :], in0=ws[:, 0:T, 0, :],
                        in1=qg[:, 0:T, D:D + H], op=mybir.AluOpType.add)
                    wse = wp.tile([128, TMAX, H], BF16, tag="wse")
                    nc.vector.tensor_tensor(
                        out=wse[:, 0:T, :],
                        in0=wsb[:, 0:T, :],
                        in1=ew_t[:],
                        op=mybir.AluOpType.mult)
                    ctb = wp.tile([128, TMAX, D + H], BF16, tag="ctb")
                    nc.scalar.activation(
                        out=ctb[:, 0:T, D:D + H], in_=wse[:, 0:T, :],
                        func=mybir.ActivationFunctionType.Exp)
                    nc.gpsimd.tensor_tensor(
                        out=ctb[:, 0:T, 0:D].rearrange("p t (e h) -> p t e h", h=H),
                        in0=kvg[:, 0:T, D:2 * D].rearrange("p t (e h) -> p t e h", h=H),
                        in1=ctb[:, 0:T, D:D + H].rearrange("p t (o h) -> p t o h", o=1)
                            .to_broadcast([128, T, HD, H]),
                        op=mybir.AluOpType.mult)

                    # sel one-hots + scatter matmuls into per-block psums
                    # (two instrs so the iota const is only TMAX/2 deep)
                    sel = wp.tile([128, 128, TMAX], BF16, tag="sel")
                    ha = T // 2
                    for s0, sn in ((0, ha), (ha, T - ha)):
                        nc.vector.tensor_tensor(
                            out=sel[:, :, s0:s0 + sn],
                            in0=oid_t[:, s0:s0 + sn]
                                .rearrange("p (o t) -> p o t", o=1)
                                .to_broadcast([128, 128, sn]),
                            in1=iot_t[:, :, 0:sn],
                            op=mybir.AluOpType.is_equal)
                    psums = {}
                    for i, b in enumerate(sg["blocks"]):
                        psums[b] = bp.tile([128, D + H], F32, tag="bps",
                                           name=f"bps{b}")[:]
                    for i, b in enumerate(sg["tile_blocks"]):
                        nc.tensor.matmul(
                            out=psums[b][:],
                            lhsT=sel[:, :, i],
                            rhs=ctb[:, i, :],
                            start=(sg["first"][b] == i),
                            stop=(sg["last"][b] == i))

                    # per-sg epilogue: divide by z, transpose, Wo -> poall
                    blocks = sg["blocks"]
                    NBk = len(blocks)
                    zr = ep.tile([128, BCAP, H], F32, tag="zr")
                    for i, b in enumerate(blocks):
                        nc.vector.tensor_scalar_add(
                            zr[:, i, :], psums[b][:, D:D + H], 1e-16)
                    nc.vector.reciprocal(zr[:, 0:NBk, :], zr[:, 0:NBk, :])
                    vals = ep.tile([128, BCAP, D], F32, tag="vals")
                    for i, b in enumerate(blocks):
                        nc.vector.tensor_tensor(
                            out=vals[:, i, :].rearrange("p (e h) -> p e h", h=H),
                            in0=psums[b][:, 0:D].rearrange("p (e h) -> p e h", h=H),
                            in1=zr[:, i, :].rearrange("p (o h) -> p o h", o=1)
                                .to_broadcast([128, HD, H]),
                            op=mybir.AluOpType.mult)
                    for i, b in enumerate(blocks):
                        tpo = tp.tile([128, 192], F32, tag="tpo", name=f"tpo{i}")
                        pt = tpo[0:D, 0:128]
                        po = tpo[:, 128:192]
                        nc.tensor.transpose(out=pt, in_=vals[:, i, :],
                                            identity=idn_t[:])
                        vT = ep.tile([D, 128], BF16, tag="vT", name=f"vT{i}")
                        nc.scalar.copy(vT[:], pt)
                        nc.tensor.matmul(out=po, lhsT=vT[:], rhs=wot_t[:],
                                         start=True, stop=True)
                        nc.scalar.copy(poall[:, b, :], po)

            mp_cm.__exit__(None, None, None)

            # ---- phase C: batched LayerNorm + residual, two half-batches.
            # xpb/out use the p-major layout: DRAM row p*NBLK + a holds the
            # data for permuted node a*128 + p (1 DMA descriptor/partition).
            with tc.tile_pool(name="fin", bufs=2) as fp:
                for b0, b1 in ((0, NBLK // 2), (NBLK // 2, NBLK)):
                    nb = b1 - b0
                    pslab = poall[:, b0:b1, :]
                    nmu = fp.tile([128, NBLK // 2 + 1], F32, tag="nmu")
                    nc.vector.tensor_reduce(
                        out=nmu[:, 0:nb], in_=pslab,
                        axis=mybir.AxisListType.X, op=mybir.AluOpType.add)
                    nc.vector.tensor_scalar_mul(
                        nmu[:, 0:nb], nmu[:, 0:nb], -1.0 / D)
                    ct = fp.tile([128, NBLK // 2 + 1, D], F32, tag="ct")
                    nc.gpsimd.tensor_tensor(
                        out=ct[:, 0:nb, :], in0=pslab,
                        in1=nmu[:, 0:nb].rearrange("p (b o) -> p b o", o=1)
                            .to_broadcast([128, nb, D]),
                        op=mybir.AluOpType.add)
                    nc.gpsimd.tensor_tensor(
                        out=ct[:, 0:nb, :], in0=ct[:, 0:nb, :],
                        in1=boc_t[:].rearrange("p (o d) -> p o d", o=1)
                            .to_broadcast([128, nb, D]),
                        op=mybir.AluOpType.add)
                    sq = fp.tile([128, NBLK // 2 + 1, D], F32, tag="sq")
                    nc.gpsimd.tensor_tensor(
                        out=sq[:, 0:nb, :], in0=ct[:, 0:nb, :],
                        in1=ct[:, 0:nb, :], op=mybir.AluOpType.mult)
                    v1 = fp.tile([128, NBLK // 2 + 1], F32, tag="v1")
                    nc.vector.tensor_reduce(
                        out=v1[:, 0:nb], in_=sq[:, 0:nb, :],
                        axis=mybir.AxisListType.X, op=mybir.AluOpType.add)
                    nc.vector.tensor_scalar(
                        out=v1[:, 0:nb], in0=v1[:, 0:nb],
                        scalar1=1.0 / D, scalar2=LN_EPS,
                        op0=mybir.AluOpType.mult, op1=mybir.AluOpType.add)
                    # rstd = exp(-0.5*ln(var+eps)): stays in the exp/ln table
                    lnv = fp.tile([128, NBLK // 2 + 1], F32, tag="lnv")
                    nc.scalar.activation(
                        out=lnv[:, 0:nb], in_=v1[:, 0:nb],
                        func=mybir.ActivationFunctionType.Ln)
                    rstd = fp.tile([128, NBLK // 2 + 1], F32, tag="rstd")
                    nc.scalar.activation(
                        out=rstd[:, 0:nb], in_=lnv[:, 0:nb], scale=-0.5,
                        func=mybir.ActivationFunctionType.Exp)
                    xb = fp.tile([128, NBLK // 2 + 1, D], F32, tag="xb")
                    nc.sync.dma_start(
                        out=xb[:, 0:nb, :],
                        in_=xpb[:].rearrange("(p a) d -> p a d", p=128)
                            [:, b0:b1, :])
                    ot = fp.tile([128, NBLK // 2 + 1, D], F32, tag="ot")
                    nc.vector.tensor_tensor(
                        out=ot[:, 0:nb, :], in0=ct[:, 0:nb, :],
                        in1=rstd[:, 0:nb].rearrange("p (b o) -> p b o", o=1)
                            .to_broadcast([128, nb, D]),
                        op=mybir.AluOpType.mult)
                    nc.gpsimd.tensor_tensor(
                        out=ot[:, 0:nb, :], in0=ot[:, 0:nb, :],
                        in1=gam_t[:].rearrange("p (o d) -> p o d", o=1)
                            .to_broadcast([128, nb, D]),
                        op=mybir.AluOpType.mult)
                    nc.gpsimd.tensor_tensor(
                        out=ot[:, 0:nb, :], in0=ot[:, 0:nb, :],
                        in1=xb[:, 0:nb, :], op=mybir.AluOpType.add)
                    nc.sync.dma_start(
                        out=out[:].rearrange("(p a) d -> p a d", p=128)
                            [:, b0:b1, :],
                        in_=ot[:, 0:nb, :])
    return nc


def kernel(x, edge_index, edge_weights, Wq, bq, Wk, bk, Wv, bv, Wo, bo,
           gamma, beta):
    x = np.asarray(x, np.float32)
    edge_index = np.asarray(edge_index)
    edge_weights = np.asarray(edge_weights, np.float32)
    origins = np.asarray(edge_index[0], np.int64)
    dests = np.asarray(edge_index[1], np.int64)

    struct, per_core = _build_structure(origins, dests, edge_weights)
    nc = _build_graph(struct)
    nc.finalize()

    # shared (replicated) host arrays.
    # v uses (e, h)-interleaved layout; k and q use it too so the d-axis
    # tree reduce groups by head with h innermost.
    vperm = (np.arange(H)[None, :] * HD + np.arange(HD)[:, None]).ravel()
    Wkf = np.asarray(Wk, np.float32)
    Wvf = np.asarray(Wv, np.float32)
    Wqf = np.asarray(Wq, np.float32)
    bkf = np.asarray(bk, np.float32)
    bvf = np.asarray(bv, np.float32)
    bqf = np.asarray(bq, np.float32)
    wkv = np.zeros((D + 1, 2 * D), np.float32)
    wkv[:D, :D] = Wkf.T[:, vperm]
    wkv[:D, D:] = Wvf.T[:, vperm]
    wkv = wkv.astype(BF16_NP)          # k/v biases fold into q.bk / boc
    # q gets H extra columns projecting x onto sum_d Wq[d,:]*bk[d] per head
    # (score = q.k_nobias + q.bk, and q.bk = x @ wqx_h + bq.bk_h)
    wq_h = np.zeros((D + 1, 2 * D), np.float32)
    wq_h[:D, 0:D] = Wqf.T[:, vperm]
    wq_h[D, 0:D] = bqf[vperm]
    for h in range(H):
        dims = np.arange(HD) + h * HD          # original k dims of head h
        wq_h[:D, D + h] = Wqf.T[:, dims] @ bkf[dims]
        wq_h[D, D + h] = bqf[dims] @ bkf[dims]
    wq_h = wq_h.astype(BF16_NP)
    wot = np.ascontiguousarray(np.asarray(Wo, np.float32).T[vperm, :])
    # bv contributes bv @ Wo.T to every output row (sum of attn = 1)
    bo2 = np.asarray(bo, np.float32) + bvf @ np.asarray(Wo, np.float32).T
    boc = np.tile((bo2 - bo2.mean())[None, :], (128, 1)).astype(np.float32)
    gam_t = np.tile(np.asarray(gamma, np.float32)[None, :], (128, 1))
    THALF = (struct["TMAX"] + 1) // 2
    iot = np.tile(np.arange(128, dtype=np.float32)[None, :, None],
                  (128, 1, THALF)).astype(BF16_NP)
    idn = np.eye(128, dtype=np.float32)

    scale = HD ** -0.5
    in_maps = []
    for c in range(NCORES):
        core = per_core[c]
        meta = _per_core_arrays(struct, core, scale)
        perm = core["perm"]
        xc = x[c * NOWN:(c + 1) * NOWN]
        xp = np.zeros((NB, D), np.float32)
        valid = perm >= 0
        xp[valid] = xc[perm[valid]]
        xTo = np.zeros((D + 1, NB), np.float32)
        xTo[:D] = xp.T
        xTo[D] = 1.0
        xTo = xTo.astype(BF16_NP)
        # kv x, dest-compacted and half-packed by table-row parity; chunk
        # ch's table rows are ranks [bounds[ch], bounds[ch+1])
        CHR = struct["chrows"]
        ucol = core["ucol"]
        bounds = core["bounds"]
        tabx = np.zeros((CH * CHR, D), np.float32)
        for ch in range(CH):
            n = int(bounds[ch + 1] - bounds[ch])
            tabx[ch * CHR:ch * CHR + n] = x[ucol[bounds[ch]:bounds[ch + 1]]]
        xTc = np.zeros((D + 1, CH * CHR), np.float32)
        xTc[:D] = tabx.T
        xTc[D] = 1.0
        xTc = xTc.astype(BF16_NP)
        # p-major: row p*NBLK + a  <-  permuted node a*128 + p
        xpb = np.ascontiguousarray(
            (xp + np.asarray(beta, np.float32)[None, :])
            .reshape(NBLK, 128, D).transpose(1, 0, 2)).reshape(NB, D)
        in_maps.append({
            "xT": xTc, "xTo": xTo, "wkv": wkv,
            "wq": wq_h, "wot": wot,
            "boc": boc, "gam": gam_t, "iot": iot, "idn": idn,
            "xpb": xpb, "meta": meta,
        })

    global LAST_SIM_NS
    if SIMULATE_COST:
        from concourse import bass_interp
        sim = bass_interp.CoreSim(nc, no_exec=True, publish_trace=False)
        sim.event_loop()
        LAST_SIM_NS = int(sim.time)

    res = run_bass_kernel_spmd(nc, in_maps, core_ids=list(range(NCORES)),
                               trace=TRACE)
    global LAST_RESULT
    LAST_RESULT = res
    full = np.zeros((N, D), np.float32)
    for c in range(NCORES):
        o = np.asarray(res.results[c]["out"])
        # p-major: row p*NBLK + a holds permuted node a*128 + p
        o = o.reshape(128, NBLK, D).transpose(1, 0, 2).reshape(NB, D)
        perm = per_core[c]["perm"]
        valid = perm >= 0
        full[c * NOWN + perm[valid]] = o[valid]
    return full


TRACE = False
SIMULATE_COST = False
LAST_RESULT = None
LAST_SIM_NS = None
